# revision 16
# baseline (speedup 1.0000x reference)
"""Multi-head attention (16 heads, L=2312, E=1024) on 8 trn2 NeuronCores.

Sharding: tensor-parallel over heads — each core computes 2 heads' full
attention (QKV proj + RoPE + softmax(QK^T)V), then 4 pipelined AllToAlls
re-shard context from head-split to interleaved sequence blocks so each
core computes a disjoint column set of the output projection while later
attention blocks are still in flight. Host reassembles the interleaved
blocks.

Key structure vs a naive port:
 - score matmuls contract over d=64 per head; the two heads live in
   disjoint SBUF partition halves, so the two matmuls land on disjoint
   PE row-groups and run concurrently (array packing).
 - context matmul is "flipped": exp-scores are the stationary operand
   (per 128-query subtile) and V^T (+ mask/ones column) streams with
   N=65, which both halves the streamed columns and yields the softmax
   denominator in the free dimension — normalization becomes lane-local
   vector work followed by a single 128x128 transpose per query tile.
 - block-0 (and early block-1) score/exp work is threaded between the
   QKV chunk matmuls so the ScalarE exp stream starts early; the output
   projection for already-arrived AllToAll groups is threaded into the
   later attention blocks so the PE never idles (keeps the HAM clock up)
   and the tail only carries the last 384 columns.
 - all DRAM inputs are laid out host-side so every DMA is 128
   contiguous rows (fast descriptor issue).

Numerics: bf16 operands with fp32 PSUM accumulation + fp32 softmax.

Self-contained: all shapes hardcoded; takes full unsharded inputs.
"""
from collections import deque

import numpy as np
import ml_dtypes

import concourse.bacc as bacc
import concourse.tile as tile
from concourse import mybir
from concourse.bass_utils import run_bass_kernel_spmd
from concourse.masks import make_identity

N_CORES = 8
L = 2312           # valid sequence length
LP = 2432          # padded to 19*128
NK = LP // 128     # 19 key tiles
E = 1024
KE = E // 128      # 8 contraction tiles over embed dim
F32 = mybir.dt.float32
BF16 = mybir.dt.bfloat16
I32 = mybir.dt.int32
SCALE = 0.125      # 1/sqrt(64)
# Schraudolph fast-exp constants (scale folded in); used on the DVE for a
# slice of the score columns to take load off the ScalarE exp stream
FE_A = float((1 << 23) * 1.4426950408889634 * SCALE)
FE_B = float(127.0 * (1 << 23) - 366392.3)
FE_C = 320        # columns [1024-FE_C : 1024) computed on the DVE

# lq blocks: (start, width); widths multiples of 128 except last (2312-2048=264)
LQB = [(0, 512), (512, 512), (1024, 512), (1536, 512), (2048, 264)]
# AllToAll groups (col_start, per-core width); group g becomes available
# after block g+1 is normalized. The last group is small to shrink the tail.
GSPEC = [(0, 84), (672, 86), (1360, 86), (2048, 48)]
NG = len(GSPEC)
# qkv N blocks over padded seq
NBLK = [(0, 256), (256, 256), (512, 512), (1024, 512), (1536, 512), (2048, 384)]
XOFF = []
_o = 0
for (_n0, _nw) in NBLK:
    XOFF.append(_o)
    _o += KE * _nw

_NC_CACHE = {}


def _subtiles(lqw):
    """(offset-in-block, width, ctx-tile-id, col-offset) per 128-query subtile."""
    out = []
    s = 0
    off = 0
    while off < lqw:
        sw = min(128, lqw - off)
        if s < 3:
            out.append((off, sw, 0, 130 * s))
        else:
            out.append((off, sw, 1, 0))
        s += 1
        off += sw
    return out


def _build():
    if "nc" in _NC_CACHE:
        return _NC_CACHE["nc"]
    nc = bacc.Bacc(
        "TRN2",
        target_bir_lowering=False,
        debug=False,
        enable_asserts=False,
        num_devices=N_CORES,
    )
    xp = nc.dram_tensor("xp", [128, KE * LP], BF16, kind="ExternalInput").ap()
    wp = nc.dram_tensor("wp", [128, 3 * KE * 128], BF16, kind="ExternalInput").ap()
    bqkv = nc.dram_tensor("bqkv", [128, 3], F32, kind="ExternalInput").ap()
    cosT = nc.dram_tensor("cosT", [128, LP], BF16, kind="ExternalInput").ap()
    sinT = nc.dram_tensor("sinT", [128, LP], BF16, kind="ExternalInput").ap()
    mskT = nc.dram_tensor("mskT", [128, NK], F32, kind="ExternalInput").ap()
    pwp = nc.dram_tensor("pwp", [128, KE * E], BF16, kind="ExternalInput").ap()
    pb = nc.dram_tensor("pb", [128, KE], F32, kind="ExternalInput").ap()
    perm = nc.dram_tensor("perm", [128, 128], BF16, kind="ExternalInput").ap()
    outT = nc.dram_tensor("outT", [128, LP], F32, kind="ExternalOutput").ap()

    with tile.TileContext(nc) as tc:
        with (
            tc.tile_pool(name="const", bufs=1) as cpool,
            tc.tile_pool(name="dram", bufs=1, space="DRAM") as dpool,
            tc.tile_pool(name="qkv", bufs=1) as qkvpool,
            tc.tile_pool(name="vaugp", bufs=1) as vaugpool,
            tc.tile_pool(name="ctxp", bufs=1) as ctxpool,
            tc.tile_pool(name="psb", bufs=12) as pspool,
            tc.tile_pool(name="cn", bufs=2) as cnpool,
            tc.tile_pool(name="rp", bufs=2) as rpool,
            tc.tile_pool(name="ps_c", bufs=1, space="PSUM") as psc,
            tc.tile_pool(name="ps_tp", bufs=1, space="PSUM") as tpp,
        ):
            identb = cpool.tile([128, 128], BF16)
            make_identity(nc, identb[:])
            pbias = cpool.tile([128, KE], F32)
            nc.gpsimd.dma_start(pbias[:], pb)
            mask_sb = cpool.tile([128, NK], F32)
            nc.gpsimd.dma_start(mask_sb[:], mskT)
            perm_sb = cpool.tile([128, 128], BF16)
            nc.gpsimd.dma_start(perm_sb[:], perm)

            Q = qkvpool.tile([128, LP], BF16)
            K = qkvpool.tile([128, LP], BF16)
            V = qkvpool.tile([128, LP], BF16)
            vaug = vaugpool.tile([128, NK, 130], BF16)
            ctxTn = ctxpool.tile([128, LP], BF16)
            cc_in = [
                dpool.tile([N_CORES, 128, w], BF16, name=f"cc_in{g}")
                for g, (_, w) in enumerate(GSPEC)
            ]
            cc_out = [
                dpool.tile([N_CORES, 128, w], BF16, name=f"cc_out{g}")
                for g, (_, w) in enumerate(GSPEC)
            ]

            # --- per-block attention state -------------------------------
            class BlockState:
                def __init__(self, lq0, lqw):
                    self.lq0, self.lqw = lq0, lqw
                    self.subs = _subtiles(lqw)
                    self.ctx = [
                        psc.tile([128, 390], F32, tag="ctxfA", name=f"ctxfA_{lq0}"),
                    ]
                    if any(ti == 1 for (_, _, ti, _) in self.subs):
                        self.ctx.append(
                            psc.tile([128, 130], F32, tag="ctxfB", name=f"ctxfB_{lq0}")
                        )
                    self.pending = []

                def init_ctx(self):
                    # PE 'start' clears has_written at bank granularity, so
                    # multiple accumulation groups per bank must be seeded by
                    # a vector memset and accumulate with start=False.
                    for ctile in self.ctx:
                        nc.vector.memset(ctile[:], 0.0)

            def ctx_mms(st, t, PSb):
                for (soff, sw, ti, coff) in st.subs:
                    ctile = st.ctx[ti]
                    for h in range(2):
                        nc.tensor.matmul(
                            ctile[0:sw, coff + 65 * h:coff + 65 * h + 65],
                            PSb[:, 512 * h + soff:512 * h + soff + sw],
                            vaug[:, t, 65 * h:65 * h + 65],
                            start=False,
                            stop=(t == NK - 1),
                        )

            def score_t(st, t, sp_pool, pop=True, psb_tag="psb", offload=False):
                SP = sp_pool.tile([128, 1024], F32, tag="sp", name="sp")
                lq0, lqw = st.lq0, st.lqw
                for h in range(2):
                    nc.tensor.matmul(
                        SP[:, 512 * h:512 * h + lqw],
                        K[64 * h:64 * h + 64, 128 * t:128 * (t + 1)],
                        Q[64 * h:64 * h + 64, lq0:lq0 + lqw],
                    )
                PSb = pspool.tile(
                    [128, 1024], BF16, tag=psb_tag, name="psb",
                    bufs=14 if psb_tag == "psb1" else None,
                )
                if lqw == 512 and offload:
                    nc.scalar.activation(
                        PSb[:, 0:1024 - FE_C], SP[:, 0:1024 - FE_C],
                        mybir.ActivationFunctionType.Exp, scale=SCALE,
                    )
                    ti = pspool.tile(
                        [128, FE_C], I32, tag="fei", name="fei", bufs=3
                    )
                    nc.vector.tensor_scalar(
                        out=ti[:], in0=SP[:, 1024 - FE_C:1024],
                        scalar1=FE_A, scalar2=FE_B,
                        op0=mybir.AluOpType.mult, op1=mybir.AluOpType.add,
                    )
                    nc.vector.tensor_copy(
                        PSb[:, 1024 - FE_C:1024], ti[:].bitcast(F32)
                    )
                elif lqw == 512:
                    nc.scalar.activation(
                        PSb[:], SP[:],
                        mybir.ActivationFunctionType.Exp, scale=SCALE,
                    )
                else:
                    for h in range(2):
                        nc.scalar.activation(
                            PSb[:, 512 * h:512 * h + lqw],
                            SP[:, 512 * h:512 * h + lqw],
                            mybir.ActivationFunctionType.Exp, scale=SCALE,
                        )
                st.pending.append((t, PSb))
                if pop:
                    while len(st.pending) >= 2:
                        ctx_mms(st, *st.pending.pop(0))

            def finish_block(st, next_st=None):
                for tp_, pb_ in st.pending:
                    ctx_mms(st, tp_, pb_)
                st.pending.clear()
                cnfs = []
                for (soff, sw, ti, coff) in st.subs:
                    ctile = st.ctx[ti]
                    CNF = cnpool.tile([128, 128], BF16, tag="cnf", name="cnf")
                    for h in range(2):
                        Rc = rpool.tile([128, 1], F32, tag="rc", name="rc")
                        nc.vector.reciprocal(
                            Rc[0:sw, :],
                            ctile[0:sw, coff + 65 * h + 64:coff + 65 * h + 65],
                        )
                        nc.vector.tensor_scalar_mul(
                            CNF[0:sw, 64 * h:64 * h + 64],
                            ctile[0:sw, coff + 65 * h:coff + 65 * h + 64],
                            Rc[0:sw, :],
                        )
                    cnfs.append(CNF)
                if next_st is not None:
                    # all ctile reads are emitted; clear the slot for the next
                    # block as early as possible so its ctx matmuls can start
                    next_st.init_ctx()
                for (soff, sw, ti, coff), CNF in zip(st.subs, cnfs):
                    TP = tpp.tile([128, 128], BF16, tag="tp", name="tp")
                    nc.tensor.transpose(
                        TP[:, 0:sw], CNF[0:sw, :], identb[0:sw, 0:sw]
                    )
                    nc.vector.tensor_copy(
                        ctxTn[:, st.lq0 + soff:st.lq0 + soff + sw], TP[:, 0:sw]
                    )

            ag = [None] * NG

            def fire_group(g, pwpool, tail=False):
                # stage ctxTn -> cc_in[g], AllToAll, unstage into SBUF
                g0, w = GSPEC[g]
                engines = (
                    [nc.gpsimd, nc.sync, nc.scalar] if tail
                    else [nc.gpsimd, nc.sync]
                )
                for j in range(N_CORES):
                    engines[j % len(engines)].dma_start(
                        cc_in[g][j], ctxTn[:, g0 + w * j:g0 + w * (j + 1)]
                    )
                nc.gpsimd.collective_compute(
                    "AllToAll",
                    mybir.AluOpType.bypass,
                    replica_groups=[list(range(N_CORES))],
                    ins=[cc_in[g].opt()],
                    outs=[cc_out[g].opt()],
                )
                ag[g] = pwpool.tile([128, KE, w], BF16, tag=f"ag{g}", name=f"ag{g}")
                for j in range(N_CORES):
                    engines[j % len(engines)].dma_start(ag[g][:, j, :], cc_out[g][j])

            # ---------------- Phase A: QKV projection + RoPE + V transpose ----
            blk0 = BlockState(*LQB[0])
            blk0.init_ctx()
            blk1 = BlockState(*LQB[1])
            sched = deque()   # (state, t) score work to thread into the chunks
            hi = {0: 0, 1: 0}
            BLK1_PHASE_A_CAP = 12
            with (
                tc.tile_pool(name="xw", bufs=1) as xwpool,
                tc.tile_pool(name="ropet", bufs=3) as rtp,
                tc.tile_pool(name="ps_a", bufs=2, space="PSUM") as psa,
                tc.tile_pool(name="ps_sw", bufs=1, space="PSUM") as psw,
                tc.tile_pool(name="ps_s0", bufs=1, space="PSUM") as spA,
            ):
                w_sb = xwpool.tile([128, 3 * KE * 128], BF16)
                nc.sync.dma_start(w_sb[:, 0:1024], wp[:, 0:1024])
                b_sb = xwpool.tile([128, 3], F32)
                nc.scalar.dma_start(b_sb[:], bqkv)
                x_sb = xwpool.tile([128, KE * LP], BF16)
                cos_sb = xwpool.tile([128, LP], BF16)
                sin_sb = xwpool.tile([128, LP], BF16)
                for bi, (n0, nw) in enumerate(NBLK):
                    nc.sync.dma_start(
                        x_sb[:, XOFF[bi]:XOFF[bi] + KE * nw],
                        xp[:, XOFF[bi]:XOFF[bi] + KE * nw],
                    )
                    if bi == 0:
                        nc.sync.dma_start(w_sb[:, 1024:3072], wp[:, 1024:3072])
                        nc.scalar.dma_start(cos_sb[:], cosT)
                        nc.scalar.dma_start(sin_sb[:], sinT)

                # mask columns of v_aug depend only on the mask DMA
                mview = mask_sb[:].rearrange("p (t o) -> p t o", o=1)
                nc.vector.tensor_copy(vaug[:, :, 64:65], mview)
                nc.vector.tensor_copy(vaug[:, :, 129:130], mview)

                def rope_chunk(T, n0, nw):
                    # rotate T[:, n0:n0+nw] in place; the 32-half swap within
                    # each head is a permutation matmul on PE.
                    swp = psw.tile([128, 512], F32, tag="swp", name=f"swp_{T.name}_{n0}")
                    nc.tensor.matmul(swp[:, :nw], perm_sb[:], T[:, n0:n0 + nw])
                    sw = rtp.tile([128, 512], BF16, tag="swap", name=f"sw_{T.name}_{n0}")
                    tmp = rtp.tile([128, 512], BF16, tag="tmp", name=f"tmp_{T.name}_{n0}")
                    nc.vector.tensor_mul(tmp[:, :nw], T[:, n0:n0 + nw], cos_sb[:, n0:n0 + nw])
                    nc.vector.tensor_mul(sw[:, :nw], swp[:, :nw], sin_sb[:, n0:n0 + nw])
                    nc.vector.tensor_add(T[:, n0:n0 + nw], tmp[:, :nw], sw[:, :nw])

                def vaug_chunk(n0, nw):
                    for t in range(n0 // 128, (n0 + nw) // 128):
                        tp = tpp.tile([128, 128], BF16, tag="tp", name="vtp")
                        nc.tensor.transpose(tp[:], V[:, 128 * t:128 * (t + 1)], identb[:])
                        nc.vector.tensor_scalar_mul(
                            vaug[:, t, 0:64], tp[:, 0:64], mask_sb[:, t:t + 1]
                        )
                        nc.vector.tensor_scalar_mul(
                            vaug[:, t, 65:129], tp[:, 64:128], mask_sb[:, t:t + 1]
                        )

                def feed(n):
                    while n and sched:
                        st_, t_ = sched.popleft()
                        if st_ is blk0:
                            score_t(st_, t_, spA)
                        else:
                            # deferred block-1 work: dedicated PSb tag so the
                            # long-lived pending tiles never share slots with
                            # block-0's stream (their reads are emitted later)
                            score_t(st_, t_, spA, pop=False, psb_tag="psb1")
                        n -= 1

                outs = [Q, K, V]
                for bi, (n0, nw) in enumerate(NBLK):
                    # newly-available score work given K roped through chunk bi-1
                    if bi >= 2:
                        kprog = n0 // 128
                        for t in range(hi[0], kprog):
                            sched.append((blk0, t))
                        hi[0] = kprog
                        if bi >= 3:  # Q[512:1024] roped after chunk 2
                            cap = min(kprog, BLK1_PHASE_A_CAP)
                            for t in range(hi[1], cap):
                                sched.append((blk1, t))
                            hi[1] = cap
                    for m in range(3):
                        feed(2)
                        ps = psa.tile([128, 512], F32, tag="qkvps")
                        for k in range(KE):
                            nc.tensor.matmul(
                                ps[:, :nw],
                                w_sb[:, 1024 * m + 128 * k:1024 * m + 128 * (k + 1)],
                                x_sb[:, XOFF[bi] + k * nw:XOFF[bi] + (k + 1) * nw],
                                start=(k == 0),
                                stop=(k == KE - 1),
                            )
                        nc.vector.tensor_scalar_add(
                            outs[m][:, n0:n0 + nw], ps[:, :nw], b_sb[:, m:m + 1]
                        )
                        if m < 2:
                            rope_chunk(outs[m], n0, nw)
                        else:
                            vaug_chunk(n0, nw)
                # flush remaining block-0 score work (K now fully roped)
                for t in range(hi[0], NK):
                    sched.append((blk0, t))
                hi[0] = NK
                feed(len(sched))

            # ---------------- Phase B: attention + AllToAll + projection ------
            with tc.tile_pool(name="pw_ag", bufs=1) as pwpool:
                # proj weights load during phase B so DMA is off the tail
                pw_sb = pwpool.tile([128, KE * E], BF16)
                nc.sync.dma_start(pw_sb[:], pwp)
                osb = pwpool.tile([128, LP], F32)

                with (
                    tc.tile_pool(name="ps_s", bufs=2, space="PSUM") as pss,
                    tc.tile_pool(name="ps_o", bufs=1, space="PSUM") as pso,
                ):
                    def proj_job(g, mE):
                        g0, w = GSPEC[g]
                        po = pso.tile([128, 96], F32, tag="po", name=f"po{g}_{mE}")
                        for k in range(KE):
                            nc.tensor.matmul(
                                po[:, 0:w],
                                pw_sb[:, 1024 * k + 128 * mE:1024 * k + 128 * (mE + 1)],
                                ag[g][:, k, :],
                                start=(k == 0),
                                stop=(k == KE - 1),
                            )
                        nc.vector.tensor_scalar_add(
                            osb[:, g0 + mE * w:g0 + (mE + 1) * w],
                            po[:, 0:w], pbias[:, mE:mE + 1],
                        )

                    def evict_group(g):
                        g0, w = GSPEC[g]
                        nc.sync.dma_start(
                            outT[:, g0:g0 + 8 * w], osb[:, g0:g0 + 8 * w]
                        )

                    prev = blk0
                    for bi in range(1, 5):
                        st = blk1 if bi == 1 else BlockState(*LQB[bi])
                        t0 = hi[1] if bi == 1 else 0
                        lead = min(4, NK - t0)
                        # pop=False: ctx matmuls into the shared ctxf slot must
                        # not be emitted before finish_block(prev) reads it
                        for t in range(t0, t0 + lead):
                            score_t(st, t, pss, pop=False, offload=True)
                        finish_block(prev, next_st=st)
                        if bi >= 2:
                            fire_group(bi - 2, pwpool)
                        for t in range(t0 + lead, NK):
                            score_t(st, t, pss, offload=True)
                        prev = st

                    # ---- tail: last block norm, small final AllToAll, then the
                    # projection; groups 0-2 overlap the final AllToAll transfer
                    finish_block(prev)
                    fire_group(3, pwpool, tail=True)
                    for g in range(NG):
                        for mE in range(KE):
                            proj_job(g, mE)
                        evict_group(g)

    nc.compile()
    _NC_CACHE["nc"] = nc
    return nc


def _prep_inputs(x, key_padding_mask, qkv_w, qkv_b, proj_w, proj_b, freqs_cos, freqs_sin):
    bf = ml_dtypes.bfloat16
    x = np.ascontiguousarray(np.asarray(x, np.float32))
    qkv_w = np.asarray(qkv_w, np.float32)
    qkv_b = np.asarray(qkv_b, np.float32)
    proj_w = np.asarray(proj_w, np.float32)
    proj_b = np.asarray(proj_b, np.float32)
    fc = np.asarray(freqs_cos, np.float32)  # [2304, 64]
    fs = np.asarray(freqs_sin, np.float32)
    mask = np.asarray(key_padding_mask)

    xpad = np.zeros((LP, E), np.float32)
    xpad[:L] = x
    xk = np.ascontiguousarray(xpad.T).reshape(KE, 128, LP)  # [k, p, n]
    xp = np.concatenate(
        [xk[:, :, n0:n0 + nw].transpose(1, 0, 2).reshape(128, KE * nw)
         for (n0, nw) in NBLK],
        axis=1,
    ).astype(bf)  # [128, KE*LP] chunk-major

    cosT = np.ones((64, LP), np.float32)
    cosT[:, 8:L] = fc.T
    cos2 = np.concatenate([cosT, cosT], axis=0).astype(bf)  # [128, LP]

    sinT = np.zeros((64, LP), np.float32)
    sinT[:, 8:L] = fs.T
    sinT[:32, :] *= -1.0  # sign of -x2 half folded into sin table
    sin2 = np.concatenate([sinT, sinT], axis=0).astype(bf)

    maskf = np.zeros((LP,), np.float32)
    maskf[:L] = mask.astype(np.float32)
    mskT = np.ascontiguousarray(maskf.reshape(NK, 128).T)  # [128, NK]

    pwT = np.ascontiguousarray(proj_w.T)  # [d, e]
    pwp = pwT.reshape(KE, 128, E).transpose(1, 0, 2).reshape(128, KE * E).astype(bf)
    pwp = np.ascontiguousarray(pwp)
    permM = np.zeros((128, 128), np.float32)  # lhsT: permM[k, m]=1 iff k==swap(m)
    for m128 in range(128):
        swp = m128 + 32 if (m128 % 64) < 32 else m128 - 32
        permM[swp, m128] = 1.0
    permM = permM.astype(bf)
    pb2 = np.ascontiguousarray(proj_b.reshape(KE, 128).T)  # [128, KE]

    in_maps = []
    for c in range(N_CORES):
        h0, h1 = 2 * c, 2 * c + 1
        rows = []
        bias_rows = []
        for sec in range(3):  # q, k, v sections of qkv_w
            for h in (h0, h1):
                sl = slice(1024 * sec + 64 * h, 1024 * sec + 64 * h + 64)
                rows.append(qkv_w[sl])
                bias_rows.append(qkv_b[sl])
        Wc = np.concatenate(rows, axis=0)           # [384, 1024]
        bc = np.concatenate(bias_rows, axis=0)      # [384]
        # m-major layout: wp[p, 1024*m + 128*k + c] = Wc[128m+c, 128k+p]
        wpc = (
            Wc.T.reshape(KE, 128, 3, 128)
            .transpose(1, 2, 0, 3)
            .reshape(128, 3 * KE * 128)
        )
        in_maps.append({
            "xp": xp,
            "wp": np.ascontiguousarray(wpc).astype(bf),
            "bqkv": np.ascontiguousarray(bc.reshape(3, 128).T),
            "cosT": cos2,
            "sinT": sin2,
            "mskT": mskT,
            "pwp": pwp,
            "pb": pb2,
            "perm": permM,
        })
    return in_maps


def _run(in_maps, trace=False):
    nc = _build()
    return run_bass_kernel_spmd(
        nc, in_maps, core_ids=list(range(N_CORES)), trace=trace
    )


def kernel(x, key_padding_mask, qkv_w, qkv_b, proj_w, proj_b, freqs_cos, freqs_sin):
    in_maps = _prep_inputs(
        x, key_padding_mask, qkv_w, qkv_b, proj_w, proj_b, freqs_cos, freqs_sin
    )
    res = _run(in_maps, trace=False)
    full = np.zeros((LP, E), np.float32)
    for c in range(N_CORES):
        r = np.asarray(res.results[c]["outT"])  # [128, LP]
        for g, (g0, w) in enumerate(GSPEC):
            block = r[:, g0:g0 + 8 * w].reshape(128, KE, w)   # [p, mE, n]
            full[g0 + w * c:g0 + w * (c + 1), :] = (
                block.transpose(2, 1, 0).reshape(w, E)
            )
    return np.ascontiguousarray(full[:L]).astype(np.float32)


# revision 18
# speedup vs baseline: 1.0769x; 1.0769x over previous
"""Multi-head attention (16 heads, L=2312, E=1024) on 8 trn2 NeuronCores.

Sharding: tensor-parallel over heads — each core computes 2 heads' full
attention (QKV proj + RoPE + softmax(QK^T)V), then 4 pipelined AllToAlls
re-shard context from head-split to interleaved sequence blocks so each
core computes a disjoint column set of the output projection while later
attention blocks are still in flight. Host reassembles the interleaved
blocks.

Key structure vs a naive port:
 - score matmuls contract over d=64 per head; the two heads live in
   disjoint SBUF partition halves, so the two matmuls land on disjoint
   PE row-groups and run concurrently (array packing).
 - context matmul is "flipped": exp-scores are the stationary operand
   (per 128-query subtile) and V^T (+ mask/ones column) streams with
   N=65, which both halves the streamed columns and yields the softmax
   denominator in the free dimension — normalization becomes lane-local
   vector work followed by a single 128x128 transpose per query tile.
 - block-0 (and early block-1) score/exp work is threaded between the
   QKV chunk matmuls so the ScalarE exp stream starts early; the output
   projection for already-arrived AllToAll groups is threaded into the
   later attention blocks so the PE never idles (keeps the HAM clock up)
   and the tail only carries the last 384 columns.
 - all DRAM inputs are laid out host-side so every DMA is 128
   contiguous rows (fast descriptor issue).

Numerics: bf16 operands with fp32 PSUM accumulation + fp32 softmax.

Self-contained: all shapes hardcoded; takes full unsharded inputs.
"""
from collections import deque

import numpy as np
import ml_dtypes

import concourse.bacc as bacc
import concourse.tile as tile
from concourse import mybir
from concourse.bass_utils import run_bass_kernel_spmd
from concourse.masks import make_identity

N_CORES = 8
L = 2312           # valid sequence length
LP = 2432          # padded to 19*128
NK = LP // 128     # 19 key tiles
E = 1024
KE = E // 128      # 8 contraction tiles over embed dim
F32 = mybir.dt.float32
BF16 = mybir.dt.bfloat16
I32 = mybir.dt.int32
SCALE = 0.125      # 1/sqrt(64)
# Schraudolph fast-exp constants (scale folded in); used on the DVE for a
# slice of the score columns to take load off the ScalarE exp stream
FE_A = float((1 << 23) * 1.4426950408889634 * SCALE)
FE_B = float(127.0 * (1 << 23) - 366392.3)
FE_C = 320        # columns [1024-FE_C : 1024) computed on the DVE

# lq blocks: (start, width); widths multiples of 128 except last (2312-2048=264)
LQB = [(0, 512), (512, 512), (1024, 512), (1536, 512), (2048, 264)]
# AllToAll groups (col_start, per-core width); group g becomes available
# after block g+1 is normalized. The last group is small to shrink the tail.
GSPEC = [(0, 84), (672, 86), (1360, 86), (2048, 48)]
NG = len(GSPEC)
# qkv N blocks over padded seq
NBLK = [(0, 256), (256, 256), (512, 512), (1024, 512), (1536, 512), (2048, 384)]
XOFF = []
_o = 0
for (_n0, _nw) in NBLK:
    XOFF.append(_o)
    _o += KE * _nw

_NC_CACHE = {}


def _subtiles(lqw):
    """(offset-in-block, width, ctx-tile-id, col-offset) per 128-query subtile."""
    out = []
    s = 0
    off = 0
    while off < lqw:
        sw = min(128, lqw - off)
        if s < 3:
            out.append((off, sw, 0, 130 * s))
        else:
            out.append((off, sw, 1, 0))
        s += 1
        off += sw
    return out


def _build():
    if "nc" in _NC_CACHE:
        return _NC_CACHE["nc"]
    nc = bacc.Bacc(
        "TRN2",
        target_bir_lowering=False,
        debug=False,
        enable_asserts=False,
        num_devices=N_CORES,
    )
    xp = nc.dram_tensor("xp", [128, KE * LP], BF16, kind="ExternalInput").ap()
    wp = nc.dram_tensor("wp", [128, 3 * KE * 128], BF16, kind="ExternalInput").ap()
    bqkv = nc.dram_tensor("bqkv", [128, 3], F32, kind="ExternalInput").ap()
    cosT = nc.dram_tensor("cosT", [128, LP], BF16, kind="ExternalInput").ap()
    sinT = nc.dram_tensor("sinT", [128, LP], BF16, kind="ExternalInput").ap()
    mskT = nc.dram_tensor("mskT", [128, NK], F32, kind="ExternalInput").ap()
    pwp = nc.dram_tensor("pwp", [128, KE * E], BF16, kind="ExternalInput").ap()
    pb = nc.dram_tensor("pb", [128, KE], F32, kind="ExternalInput").ap()
    perm = nc.dram_tensor("perm", [128, 128], BF16, kind="ExternalInput").ap()
    outT = nc.dram_tensor("outT", [128, LP], F32, kind="ExternalOutput").ap()

    with tile.TileContext(nc) as tc:
        with (
            tc.tile_pool(name="const", bufs=1) as cpool,
            tc.tile_pool(name="dram", bufs=1, space="DRAM") as dpool,
            tc.tile_pool(name="qkv", bufs=1) as qkvpool,
            tc.tile_pool(name="vaugp", bufs=1) as vaugpool,
            tc.tile_pool(name="ctxp", bufs=1) as ctxpool,
            tc.tile_pool(name="psb", bufs=12) as pspool,
            tc.tile_pool(name="cn", bufs=2) as cnpool,
            tc.tile_pool(name="rp", bufs=2) as rpool,
            tc.tile_pool(name="ps_c", bufs=1, space="PSUM") as psc,
            tc.tile_pool(name="ps_tp", bufs=1, space="PSUM") as tpp,
        ):
            identb = cpool.tile([128, 128], BF16)
            make_identity(nc, identb[:])
            pbias = cpool.tile([128, KE], F32)
            nc.gpsimd.dma_start(pbias[:], pb)
            mask_sb = cpool.tile([128, NK], F32)
            nc.gpsimd.dma_start(mask_sb[:], mskT)
            perm_sb = cpool.tile([128, 128], BF16)
            nc.gpsimd.dma_start(perm_sb[:], perm)

            Q = qkvpool.tile([128, LP], BF16)
            K = qkvpool.tile([128, LP], BF16)
            V = qkvpool.tile([128, LP], BF16)
            vaug = vaugpool.tile([128, NK, 130], BF16)
            ctxTn = ctxpool.tile([128, LP], BF16)
            cc_in = [
                dpool.tile([N_CORES, 128, w], BF16, name=f"cc_in{g}")
                for g, (_, w) in enumerate(GSPEC)
            ]
            cc_out = [
                dpool.tile([N_CORES, 128, w], BF16, name=f"cc_out{g}")
                for g, (_, w) in enumerate(GSPEC)
            ]

            # --- per-block attention state -------------------------------
            class BlockState:
                def __init__(self, lq0, lqw):
                    self.lq0, self.lqw = lq0, lqw
                    self.subs = _subtiles(lqw)
                    self.ctx = [
                        psc.tile([128, 390], F32, tag="ctxfA", name=f"ctxfA_{lq0}"),
                    ]
                    if any(ti == 1 for (_, _, ti, _) in self.subs):
                        self.ctx.append(
                            psc.tile([128, 130], F32, tag="ctxfB", name=f"ctxfB_{lq0}")
                        )
                    self.pending = []

                def init_ctx(self):
                    # PE 'start' clears has_written at bank granularity, so
                    # multiple accumulation groups per bank must be seeded by
                    # a vector memset and accumulate with start=False.
                    for ctile in self.ctx:
                        nc.vector.memset(ctile[:], 0.0)

            def ctx_mms(st, t, PSb):
                for (soff, sw, ti, coff) in st.subs:
                    ctile = st.ctx[ti]
                    for h in range(2):
                        nc.tensor.matmul(
                            ctile[0:sw, coff + 65 * h:coff + 65 * h + 65],
                            PSb[:, 512 * h + soff:512 * h + soff + sw],
                            vaug[:, t, 65 * h:65 * h + 65],
                            start=False,
                            stop=(t == NK - 1),
                        )

            def score_t(st, t, sp_pool, pop=True, psb_tag="psb", offload=False):
                SP = sp_pool.tile([128, 1024], F32, tag="sp", name="sp")
                lq0, lqw = st.lq0, st.lqw
                for h in range(2):
                    nc.tensor.matmul(
                        SP[:, 512 * h:512 * h + lqw],
                        K[64 * h:64 * h + 64, 128 * t:128 * (t + 1)],
                        Q[64 * h:64 * h + 64, lq0:lq0 + lqw],
                    )
                PSb = pspool.tile(
                    [128, 1024], BF16, tag=psb_tag, name="psb",
                    bufs=14 if psb_tag == "psb1" else None,
                )
                if lqw == 512 and offload:
                    nc.scalar.activation(
                        PSb[:, 0:1024 - FE_C], SP[:, 0:1024 - FE_C],
                        mybir.ActivationFunctionType.Exp, scale=SCALE,
                    )
                    ti = pspool.tile(
                        [128, FE_C], I32, tag="fei", name="fei", bufs=3
                    )
                    nc.vector.tensor_scalar(
                        out=ti[:], in0=SP[:, 1024 - FE_C:1024],
                        scalar1=FE_A, scalar2=FE_B,
                        op0=mybir.AluOpType.mult, op1=mybir.AluOpType.add,
                    )
                    nc.vector.tensor_copy(
                        PSb[:, 1024 - FE_C:1024], ti[:].bitcast(F32)
                    )
                elif lqw == 512:
                    nc.scalar.activation(
                        PSb[:], SP[:],
                        mybir.ActivationFunctionType.Exp, scale=SCALE,
                    )
                else:
                    for h in range(2):
                        nc.scalar.activation(
                            PSb[:, 512 * h:512 * h + lqw],
                            SP[:, 512 * h:512 * h + lqw],
                            mybir.ActivationFunctionType.Exp, scale=SCALE,
                        )
                st.pending.append((t, PSb))
                if pop:
                    while len(st.pending) >= 2:
                        ctx_mms(st, *st.pending.pop(0))

            def finish_block(st, next_st=None):
                for tp_, pb_ in st.pending:
                    ctx_mms(st, tp_, pb_)
                st.pending.clear()
                cnfs = []
                for (soff, sw, ti, coff) in st.subs:
                    ctile = st.ctx[ti]
                    CNF = cnpool.tile([128, 128], BF16, tag="cnf", name="cnf")
                    for h in range(2):
                        Rc = rpool.tile([128, 1], F32, tag="rc", name="rc")
                        nc.vector.reciprocal(
                            Rc[0:sw, :],
                            ctile[0:sw, coff + 65 * h + 64:coff + 65 * h + 65],
                        )
                        nc.vector.tensor_scalar_mul(
                            CNF[0:sw, 64 * h:64 * h + 64],
                            ctile[0:sw, coff + 65 * h:coff + 65 * h + 64],
                            Rc[0:sw, :],
                        )
                    cnfs.append(CNF)
                if next_st is not None:
                    # all ctile reads are emitted; clear the slot for the next
                    # block as early as possible so its ctx matmuls can start
                    next_st.init_ctx()
                for (soff, sw, ti, coff), CNF in zip(st.subs, cnfs):
                    TP = tpp.tile([128, 128], BF16, tag="tp", name="tp")
                    nc.tensor.transpose(
                        TP[:, 0:sw], CNF[0:sw, :], identb[0:sw, 0:sw]
                    )
                    nc.vector.tensor_copy(
                        ctxTn[:, st.lq0 + soff:st.lq0 + soff + sw], TP[:, 0:sw]
                    )

            ag = [None] * NG

            def fire_group(g, pwpool, tail=False):
                # stage ctxTn -> cc_in[g], AllToAll, unstage into SBUF
                g0, w = GSPEC[g]
                engines = (
                    [nc.gpsimd, nc.sync, nc.scalar] if tail
                    else [nc.gpsimd, nc.sync]
                )
                for j in range(N_CORES):
                    engines[j % len(engines)].dma_start(
                        cc_in[g][j], ctxTn[:, g0 + w * j:g0 + w * (j + 1)]
                    )
                nc.gpsimd.collective_compute(
                    "AllToAll",
                    mybir.AluOpType.bypass,
                    replica_groups=[list(range(N_CORES))],
                    ins=[cc_in[g].opt()],
                    outs=[cc_out[g].opt()],
                )
                ag[g] = pwpool.tile([128, KE, w], BF16, tag=f"ag{g}", name=f"ag{g}")
                for j in range(N_CORES):
                    engines[j % len(engines)].dma_start(ag[g][:, j, :], cc_out[g][j])

            # ---------------- Phase A: QKV projection + RoPE + V transpose ----
            blk0 = BlockState(*LQB[0])
            blk0.init_ctx()
            blk1 = BlockState(*LQB[1])
            sched = deque()   # (state, t) score work to thread into the chunks
            hi = {0: 0, 1: 0}
            BLK1_PHASE_A_CAP = 12
            with (
                tc.tile_pool(name="xw", bufs=1) as xwpool,
                tc.tile_pool(name="ropet", bufs=3) as rtp,
                tc.tile_pool(name="ps_a", bufs=2, space="PSUM") as psa,
                tc.tile_pool(name="ps_sw", bufs=1, space="PSUM") as psw,
                tc.tile_pool(name="ps_s0", bufs=1, space="PSUM") as spA,
            ):
                w_sb = xwpool.tile([128, 3 * KE * 128], BF16)
                nc.sync.dma_start(w_sb[:, 0:1024], wp[:, 0:1024])
                b_sb = xwpool.tile([128, 3], F32)
                nc.scalar.dma_start(b_sb[:], bqkv)
                x_sb = xwpool.tile([128, KE * LP], BF16)
                cos_sb = xwpool.tile([128, LP], BF16)
                sin_sb = xwpool.tile([128, LP], BF16)
                for bi, (n0, nw) in enumerate(NBLK):
                    nc.sync.dma_start(
                        x_sb[:, XOFF[bi]:XOFF[bi] + KE * nw],
                        xp[:, XOFF[bi]:XOFF[bi] + KE * nw],
                    )
                    if bi == 0:
                        nc.sync.dma_start(w_sb[:, 1024:3072], wp[:, 1024:3072])
                        nc.scalar.dma_start(cos_sb[:], cosT)
                        nc.scalar.dma_start(sin_sb[:], sinT)

                # mask columns of v_aug depend only on the mask DMA
                mview = mask_sb[:].rearrange("p (t o) -> p t o", o=1)
                nc.vector.tensor_copy(vaug[:, :, 64:65], mview)
                nc.vector.tensor_copy(vaug[:, :, 129:130], mview)

                def rope_chunk(T, n0, nw):
                    # rotate T[:, n0:n0+nw] in place; the 32-half swap within
                    # each head is a permutation matmul on PE.
                    swp = psw.tile([128, 512], F32, tag="swp", name=f"swp_{T.name}_{n0}")
                    nc.tensor.matmul(swp[:, :nw], perm_sb[:], T[:, n0:n0 + nw])
                    sw = rtp.tile([128, 512], BF16, tag="swap", name=f"sw_{T.name}_{n0}")
                    tmp = rtp.tile([128, 512], BF16, tag="tmp", name=f"tmp_{T.name}_{n0}")
                    nc.vector.tensor_mul(tmp[:, :nw], T[:, n0:n0 + nw], cos_sb[:, n0:n0 + nw])
                    nc.vector.tensor_mul(sw[:, :nw], swp[:, :nw], sin_sb[:, n0:n0 + nw])
                    nc.vector.tensor_add(T[:, n0:n0 + nw], tmp[:, :nw], sw[:, :nw])

                def vaug_chunk(n0, nw):
                    for t in range(n0 // 128, (n0 + nw) // 128):
                        tp = tpp.tile([128, 128], BF16, tag="tp", name="vtp")
                        nc.tensor.transpose(tp[:], V[:, 128 * t:128 * (t + 1)], identb[:])
                        nc.vector.tensor_scalar_mul(
                            vaug[:, t, 0:64], tp[:, 0:64], mask_sb[:, t:t + 1]
                        )
                        nc.vector.tensor_scalar_mul(
                            vaug[:, t, 65:129], tp[:, 64:128], mask_sb[:, t:t + 1]
                        )

                def feed(n):
                    while n and sched:
                        st_, t_ = sched.popleft()
                        if st_ is blk0:
                            score_t(st_, t_, spA)
                        else:
                            # deferred block-1 work: dedicated PSb tag so the
                            # long-lived pending tiles never share slots with
                            # block-0's stream (their reads are emitted later)
                            score_t(st_, t_, spA, pop=False, psb_tag="psb1")
                        n -= 1

                outs = [Q, K, V]
                for bi, (n0, nw) in enumerate(NBLK):
                    # newly-available score work given K roped through chunk bi-1
                    if bi >= 2:
                        kprog = n0 // 128
                        for t in range(hi[0], kprog):
                            sched.append((blk0, t))
                        hi[0] = kprog
                        if bi >= 3:  # Q[512:1024] roped after chunk 2
                            cap = min(kprog, BLK1_PHASE_A_CAP)
                            for t in range(hi[1], cap):
                                sched.append((blk1, t))
                            hi[1] = cap
                    for m in range(3):
                        feed(2)
                        ps = psa.tile([128, 512], F32, tag="qkvps")
                        for k in range(KE):
                            nc.tensor.matmul(
                                ps[:, :nw],
                                w_sb[:, 1024 * m + 128 * k:1024 * m + 128 * (k + 1)],
                                x_sb[:, XOFF[bi] + k * nw:XOFF[bi] + (k + 1) * nw],
                                start=(k == 0),
                                stop=(k == KE - 1),
                            )
                        nc.vector.tensor_scalar_add(
                            outs[m][:, n0:n0 + nw], ps[:, :nw], b_sb[:, m:m + 1]
                        )
                        if m < 2:
                            rope_chunk(outs[m], n0, nw)
                        else:
                            vaug_chunk(n0, nw)
                # flush remaining block-0 score work (K now fully roped)
                for t in range(hi[0], NK):
                    sched.append((blk0, t))
                hi[0] = NK
                feed(len(sched))

            # ---------------- Phase B: attention + AllToAll + projection ------
            with tc.tile_pool(name="pw_ag", bufs=1) as pwpool:
                # proj weights load during phase B so DMA is off the tail
                pw_sb = pwpool.tile([128, KE * E], BF16)
                nc.sync.dma_start(pw_sb[:], pwp)
                osb = pwpool.tile([128, LP], F32)

                with (
                    tc.tile_pool(name="ps_s", bufs=2, space="PSUM") as pss,
                    tc.tile_pool(name="ps_o", bufs=1, space="PSUM") as pso,
                ):
                    def warm_mm():
                        # dependency-free matmul into a scratch bank: keeps the
                        # PE busy enough that HAM holds the clock up through
                        # the exp-paced attention blocks
                        po = pso.tile([128, 512], F32, tag="po", name="warm")
                        nc.tensor.matmul(
                            po[:], K[0:64, 0:128], Q[0:64, 0:512],
                        )

                    def proj_job(g, mE):
                        g0, w = GSPEC[g]
                        po = pso.tile([128, 512], F32, tag="po", name=f"po{g}_{mE}")
                        for k in range(KE):
                            nc.tensor.matmul(
                                po[:, 0:w],
                                pw_sb[:, 1024 * k + 128 * mE:1024 * k + 128 * (mE + 1)],
                                ag[g][:, k, :],
                                start=(k == 0),
                                stop=(k == KE - 1),
                            )
                        nc.vector.tensor_scalar_add(
                            osb[:, g0 + mE * w:g0 + (mE + 1) * w],
                            po[:, 0:w], pbias[:, mE:mE + 1],
                        )

                    def evict_group(g):
                        g0, w = GSPEC[g]
                        nc.sync.dma_start(
                            outT[:, g0:g0 + 8 * w], osb[:, g0:g0 + 8 * w]
                        )

                    prev = blk0
                    for bi in range(1, 5):
                        st = blk1 if bi == 1 else BlockState(*LQB[bi])
                        t0 = hi[1] if bi == 1 else 0
                        lead = min(4, NK - t0)
                        # pop=False: ctx matmuls into the shared ctxf slot must
                        # not be emitted before finish_block(prev) reads it
                        for t in range(t0, t0 + lead):
                            score_t(st, t, pss, pop=False)
                        finish_block(prev, next_st=st)
                        if bi >= 2:
                            fire_group(bi - 2, pwpool)
                        filler = deque(
                            (0, mE) for mE in range(KE)
                        ) if bi == 4 else deque()
                        for t in range(t0 + lead, NK):
                            score_t(st, t, pss)
                            if bi == 4 and filler and t % 2 == 1 and t >= 3:
                                proj_job(*filler.popleft())
                            else:
                                warm_mm()
                        while filler:
                            proj_job(*filler.popleft())
                        prev = st

                    # ---- tail: last block norm, small final AllToAll, then the
                    # projection; groups 0-2 overlap the final AllToAll transfer
                    finish_block(prev)
                    fire_group(3, pwpool, tail=True)
                    evict_group(0)
                    for g in range(1, NG):
                        for mE in range(KE):
                            proj_job(g, mE)
                        evict_group(g)

    nc.compile()
    _NC_CACHE["nc"] = nc
    return nc


def _prep_inputs(x, key_padding_mask, qkv_w, qkv_b, proj_w, proj_b, freqs_cos, freqs_sin):
    bf = ml_dtypes.bfloat16
    x = np.ascontiguousarray(np.asarray(x, np.float32))
    qkv_w = np.asarray(qkv_w, np.float32)
    qkv_b = np.asarray(qkv_b, np.float32)
    proj_w = np.asarray(proj_w, np.float32)
    proj_b = np.asarray(proj_b, np.float32)
    fc = np.asarray(freqs_cos, np.float32)  # [2304, 64]
    fs = np.asarray(freqs_sin, np.float32)
    mask = np.asarray(key_padding_mask)

    xpad = np.zeros((LP, E), np.float32)
    xpad[:L] = x
    xk = np.ascontiguousarray(xpad.T).reshape(KE, 128, LP)  # [k, p, n]
    xp = np.concatenate(
        [xk[:, :, n0:n0 + nw].transpose(1, 0, 2).reshape(128, KE * nw)
         for (n0, nw) in NBLK],
        axis=1,
    ).astype(bf)  # [128, KE*LP] chunk-major

    cosT = np.ones((64, LP), np.float32)
    cosT[:, 8:L] = fc.T
    cos2 = np.concatenate([cosT, cosT], axis=0).astype(bf)  # [128, LP]

    sinT = np.zeros((64, LP), np.float32)
    sinT[:, 8:L] = fs.T
    sinT[:32, :] *= -1.0  # sign of -x2 half folded into sin table
    sin2 = np.concatenate([sinT, sinT], axis=0).astype(bf)

    maskf = np.zeros((LP,), np.float32)
    maskf[:L] = mask.astype(np.float32)
    mskT = np.ascontiguousarray(maskf.reshape(NK, 128).T)  # [128, NK]

    pwT = np.ascontiguousarray(proj_w.T)  # [d, e]
    pwp = pwT.reshape(KE, 128, E).transpose(1, 0, 2).reshape(128, KE * E).astype(bf)
    pwp = np.ascontiguousarray(pwp)
    permM = np.zeros((128, 128), np.float32)  # lhsT: permM[k, m]=1 iff k==swap(m)
    for m128 in range(128):
        swp = m128 + 32 if (m128 % 64) < 32 else m128 - 32
        permM[swp, m128] = 1.0
    permM = permM.astype(bf)
    pb2 = np.ascontiguousarray(proj_b.reshape(KE, 128).T)  # [128, KE]

    in_maps = []
    for c in range(N_CORES):
        h0, h1 = 2 * c, 2 * c + 1
        rows = []
        bias_rows = []
        for sec in range(3):  # q, k, v sections of qkv_w
            for h in (h0, h1):
                sl = slice(1024 * sec + 64 * h, 1024 * sec + 64 * h + 64)
                rows.append(qkv_w[sl])
                bias_rows.append(qkv_b[sl])
        Wc = np.concatenate(rows, axis=0)           # [384, 1024]
        bc = np.concatenate(bias_rows, axis=0)      # [384]
        # m-major layout: wp[p, 1024*m + 128*k + c] = Wc[128m+c, 128k+p]
        wpc = (
            Wc.T.reshape(KE, 128, 3, 128)
            .transpose(1, 2, 0, 3)
            .reshape(128, 3 * KE * 128)
        )
        in_maps.append({
            "xp": xp,
            "wp": np.ascontiguousarray(wpc).astype(bf),
            "bqkv": np.ascontiguousarray(bc.reshape(3, 128).T),
            "cosT": cos2,
            "sinT": sin2,
            "mskT": mskT,
            "pwp": pwp,
            "pb": pb2,
            "perm": permM,
        })
    return in_maps


def _run(in_maps, trace=False):
    nc = _build()
    return run_bass_kernel_spmd(
        nc, in_maps, core_ids=list(range(N_CORES)), trace=trace
    )


def kernel(x, key_padding_mask, qkv_w, qkv_b, proj_w, proj_b, freqs_cos, freqs_sin):
    in_maps = _prep_inputs(
        x, key_padding_mask, qkv_w, qkv_b, proj_w, proj_b, freqs_cos, freqs_sin
    )
    res = _run(in_maps, trace=False)
    full = np.zeros((LP, E), np.float32)
    for c in range(N_CORES):
        r = np.asarray(res.results[c]["outT"])  # [128, LP]
        for g, (g0, w) in enumerate(GSPEC):
            block = r[:, g0:g0 + 8 * w].reshape(128, KE, w)   # [p, mE, n]
            full[g0 + w * c:g0 + w * (c + 1), :] = (
                block.transpose(2, 1, 0).reshape(w, E)
            )
    return np.ascontiguousarray(full[:L]).astype(np.float32)


# revision 20
# speedup vs baseline: 1.1487x; 1.0666x over previous
"""Multi-head attention (16 heads, L=2312, E=1024) on 8 trn2 NeuronCores.

Sharding: tensor-parallel over heads — each core computes 2 heads' full
attention (QKV proj + RoPE + softmax(QK^T)V), then 4 pipelined AllToAlls
re-shard context from head-split to interleaved sequence blocks so each
core computes a disjoint column set of the output projection while later
attention blocks are still in flight. Host reassembles the interleaved
blocks.

Key structure vs a naive port:
 - score matmuls contract over d=64 per head; the two heads live in
   disjoint SBUF partition halves, so the two matmuls land on disjoint
   PE row-groups and run concurrently (array packing).
 - context matmul is "flipped": exp-scores are the stationary operand
   (per 128-query subtile) and V^T (+ mask/ones column) streams with
   N=65, which both halves the streamed columns and yields the softmax
   denominator in the free dimension — normalization becomes lane-local
   vector work followed by a single 128x128 transpose per query tile.
 - block-0 (and early block-1) score/exp work is threaded between the
   QKV chunk matmuls so the ScalarE exp stream starts early; the output
   projection for already-arrived AllToAll groups is threaded into the
   later attention blocks so the PE never idles (keeps the HAM clock up)
   and the tail only carries the last 384 columns.
 - all DRAM inputs are laid out host-side so every DMA is 128
   contiguous rows (fast descriptor issue).

Numerics: bf16 operands with fp32 PSUM accumulation + fp32 softmax.

Self-contained: all shapes hardcoded; takes full unsharded inputs.
"""
from collections import deque

import numpy as np
import ml_dtypes

import concourse.bacc as bacc
import concourse.tile as tile
from concourse import mybir
from concourse.bass_utils import run_bass_kernel_spmd
from concourse.masks import make_identity

N_CORES = 8
L = 2312           # valid sequence length
LP = 2432          # padded to 19*128
NK = LP // 128     # 19 key tiles
E = 1024
KE = E // 128      # 8 contraction tiles over embed dim
F32 = mybir.dt.float32
BF16 = mybir.dt.bfloat16
I32 = mybir.dt.int32
SCALE = 0.125      # 1/sqrt(64)
# Schraudolph fast-exp constants (scale folded in); used on the DVE for a
# slice of the score columns to take load off the ScalarE exp stream
FE_A = float((1 << 23) * 1.4426950408889634 * SCALE)
FE_B = float(127.0 * (1 << 23) - 366392.3)
FE_C = 320        # columns [1024-FE_C : 1024) computed on the DVE

# lq blocks: (start, width); widths multiples of 128 except last (2312-2048=264)
LQB = [(0, 512), (512, 512), (1024, 512), (1536, 512), (2048, 264)]
# AllToAll groups (col_start, per-core width); group g becomes available
# after block g+1 is normalized. The last group is small to shrink the tail.
GSPEC = [(0, 84), (672, 86), (1360, 86), (2048, 48)]
NG = len(GSPEC)
# qkv N blocks over padded seq
NBLK = [(0, 256), (256, 256), (512, 512), (1024, 512), (1536, 512), (2048, 384)]
XOFF = []
_o = 0
for (_n0, _nw) in NBLK:
    XOFF.append(_o)
    _o += KE * _nw

_NC_CACHE = {}


def _subtiles(lqw):
    """(offset-in-block, width, ctx-tile-id, col-offset) per 128-query subtile."""
    out = []
    s = 0
    off = 0
    while off < lqw:
        sw = min(128, lqw - off)
        if s < 3:
            out.append((off, sw, 0, 130 * s))
        else:
            out.append((off, sw, 1, 0))
        s += 1
        off += sw
    return out


def _build():
    if "nc" in _NC_CACHE:
        return _NC_CACHE["nc"]
    nc = bacc.Bacc(
        "TRN2",
        target_bir_lowering=False,
        debug=False,
        enable_asserts=False,
        num_devices=N_CORES,
    )
    xp = nc.dram_tensor("xp", [128, KE * LP], BF16, kind="ExternalInput").ap()
    wp = nc.dram_tensor("wp", [128, 3 * KE * 128], BF16, kind="ExternalInput").ap()
    bqkv = nc.dram_tensor("bqkv", [128, 3], F32, kind="ExternalInput").ap()
    cosT = nc.dram_tensor("cosT", [128, LP], BF16, kind="ExternalInput").ap()
    sinT = nc.dram_tensor("sinT", [128, LP], BF16, kind="ExternalInput").ap()
    mskT = nc.dram_tensor("mskT", [128, NK], F32, kind="ExternalInput").ap()
    pwp = nc.dram_tensor("pwp", [128, KE * E], BF16, kind="ExternalInput").ap()
    pb = nc.dram_tensor("pb", [128, KE], F32, kind="ExternalInput").ap()
    perm = nc.dram_tensor("perm", [128, 128], BF16, kind="ExternalInput").ap()
    outT = nc.dram_tensor("outT", [128, LP], F32, kind="ExternalOutput").ap()

    with tile.TileContext(nc) as tc:
        with (
            tc.tile_pool(name="const", bufs=1) as cpool,
            tc.tile_pool(name="dram", bufs=1, space="DRAM") as dpool,
            tc.tile_pool(name="qkv", bufs=1) as qkvpool,
            tc.tile_pool(name="vaugp", bufs=1) as vaugpool,
            tc.tile_pool(name="ctxp", bufs=1) as ctxpool,
            tc.tile_pool(name="psb", bufs=12) as pspool,
            tc.tile_pool(name="cn", bufs=2) as cnpool,
            tc.tile_pool(name="rp", bufs=2) as rpool,
            tc.tile_pool(name="ps_c", bufs=1, space="PSUM") as psc,
            tc.tile_pool(name="ps_tp", bufs=1, space="PSUM") as tpp,
        ):
            identb = cpool.tile([128, 128], BF16)
            make_identity(nc, identb[:])
            pbias = cpool.tile([128, KE], F32)
            nc.gpsimd.dma_start(pbias[:], pb)
            mask_sb = cpool.tile([128, NK], F32)
            nc.gpsimd.dma_start(mask_sb[:], mskT)
            perm_sb = cpool.tile([128, 128], BF16)
            nc.gpsimd.dma_start(perm_sb[:], perm)

            Q = qkvpool.tile([128, LP], BF16)
            K = qkvpool.tile([128, LP], BF16)
            V = qkvpool.tile([128, LP], BF16)
            vaug = vaugpool.tile([128, NK, 130], BF16)
            ctxTn = ctxpool.tile([128, LP], BF16)
            cc_in = [
                dpool.tile([N_CORES, 128, w], BF16, name=f"cc_in{g}")
                for g, (_, w) in enumerate(GSPEC)
            ]
            cc_out = [
                dpool.tile([N_CORES, 128, w], BF16, name=f"cc_out{g}")
                for g, (_, w) in enumerate(GSPEC)
            ]

            # --- per-block attention state -------------------------------
            class BlockState:
                def __init__(self, lq0, lqw):
                    self.lq0, self.lqw = lq0, lqw
                    self.subs = _subtiles(lqw)
                    self.ctx = [
                        psc.tile([128, 390], F32, tag="ctxfA", name=f"ctxfA_{lq0}"),
                    ]
                    if any(ti == 1 for (_, _, ti, _) in self.subs):
                        self.ctx.append(
                            psc.tile([128, 130], F32, tag="ctxfB", name=f"ctxfB_{lq0}")
                        )
                    self.pending = []

                def init_ctx(self):
                    # PE 'start' clears has_written at bank granularity, so
                    # multiple accumulation groups per bank must be seeded by
                    # a vector memset and accumulate with start=False.
                    for ctile in self.ctx:
                        nc.vector.memset(ctile[:], 0.0)

            def ctx_mms(st, t, PSb):
                for (soff, sw, ti, coff) in st.subs:
                    ctile = st.ctx[ti]
                    for h in range(2):
                        nc.tensor.matmul(
                            ctile[0:sw, coff + 65 * h:coff + 65 * h + 65],
                            PSb[:, 512 * h + soff:512 * h + soff + sw],
                            vaug[:, t, 65 * h:65 * h + 65],
                            start=False,
                            stop=(t == NK - 1),
                        )

            def score_t(st, t, sp_pool, pop=True, psb_tag="psb", offload=False):
                SP = sp_pool.tile([128, 1024], F32, tag="sp", name="sp")
                lq0, lqw = st.lq0, st.lqw
                for h in range(2):
                    nc.tensor.matmul(
                        SP[:, 512 * h:512 * h + lqw],
                        K[64 * h:64 * h + 64, 128 * t:128 * (t + 1)],
                        Q[64 * h:64 * h + 64, lq0:lq0 + lqw],
                    )
                PSb = pspool.tile(
                    [128, 1024], BF16, tag=psb_tag, name="psb",
                    bufs=14 if psb_tag == "psb1" else None,
                )
                if lqw == 512 and offload:
                    nc.scalar.activation(
                        PSb[:, 0:1024 - FE_C], SP[:, 0:1024 - FE_C],
                        mybir.ActivationFunctionType.Exp, scale=SCALE,
                    )
                    ti = pspool.tile(
                        [128, FE_C], I32, tag="fei", name="fei", bufs=3
                    )
                    nc.vector.tensor_scalar(
                        out=ti[:], in0=SP[:, 1024 - FE_C:1024],
                        scalar1=FE_A, scalar2=FE_B,
                        op0=mybir.AluOpType.mult, op1=mybir.AluOpType.add,
                    )
                    nc.vector.tensor_copy(
                        PSb[:, 1024 - FE_C:1024], ti[:].bitcast(F32)
                    )
                elif lqw == 512:
                    nc.scalar.activation(
                        PSb[:], SP[:],
                        mybir.ActivationFunctionType.Exp, scale=SCALE,
                    )
                else:
                    for h in range(2):
                        nc.scalar.activation(
                            PSb[:, 512 * h:512 * h + lqw],
                            SP[:, 512 * h:512 * h + lqw],
                            mybir.ActivationFunctionType.Exp, scale=SCALE,
                        )
                st.pending.append((t, PSb))
                if pop:
                    while len(st.pending) >= 2:
                        ctx_mms(st, *st.pending.pop(0))

            def finish_block(st, next_st=None):
                for tp_, pb_ in st.pending:
                    ctx_mms(st, tp_, pb_)
                st.pending.clear()
                cnfs = []
                for (soff, sw, ti, coff) in st.subs:
                    ctile = st.ctx[ti]
                    CNF = cnpool.tile([128, 128], BF16, tag="cnf", name="cnf")
                    for h in range(2):
                        Rc = rpool.tile([128, 1], F32, tag="rc", name="rc")
                        nc.vector.reciprocal(
                            Rc[0:sw, :],
                            ctile[0:sw, coff + 65 * h + 64:coff + 65 * h + 65],
                        )
                        nc.vector.tensor_scalar_mul(
                            CNF[0:sw, 64 * h:64 * h + 64],
                            ctile[0:sw, coff + 65 * h:coff + 65 * h + 64],
                            Rc[0:sw, :],
                        )
                    cnfs.append(CNF)
                if next_st is not None:
                    # all ctile reads are emitted; clear the slot for the next
                    # block as early as possible so its ctx matmuls can start
                    next_st.init_ctx()
                for (soff, sw, ti, coff), CNF in zip(st.subs, cnfs):
                    TP = tpp.tile([128, 128], BF16, tag="tp", name="tp")
                    nc.tensor.transpose(
                        TP[:, 0:sw], CNF[0:sw, :], identb[0:sw, 0:sw]
                    )
                    nc.vector.tensor_copy(
                        ctxTn[:, st.lq0 + soff:st.lq0 + soff + sw], TP[:, 0:sw]
                    )

            ag = [None] * NG

            def fire_group(g, pwpool, tail=False):
                # stage ctxTn -> cc_in[g], AllToAll, unstage into SBUF
                g0, w = GSPEC[g]
                engines = (
                    [nc.gpsimd, nc.sync, nc.scalar] if tail
                    else [nc.gpsimd, nc.sync]
                )
                for j in range(N_CORES):
                    engines[j % len(engines)].dma_start(
                        cc_in[g][j], ctxTn[:, g0 + w * j:g0 + w * (j + 1)]
                    )
                nc.gpsimd.collective_compute(
                    "AllToAll",
                    mybir.AluOpType.bypass,
                    replica_groups=[list(range(N_CORES))],
                    ins=[cc_in[g].opt()],
                    outs=[cc_out[g].opt()],
                )
                ag[g] = pwpool.tile([128, KE, w], BF16, tag=f"ag{g}", name=f"ag{g}")
                for j in range(N_CORES):
                    engines[j % len(engines)].dma_start(ag[g][:, j, :], cc_out[g][j])

            # ---------------- Phase A: QKV projection + RoPE + V transpose ----
            blk0 = BlockState(*LQB[0])
            blk0.init_ctx()
            blk1 = BlockState(*LQB[1])
            sched = deque()   # (state, t) score work to thread into the chunks
            hi = {0: 0, 1: 0}
            BLK1_PHASE_A_CAP = 12
            with (
                tc.tile_pool(name="xw", bufs=1) as xwpool,
                tc.tile_pool(name="ropet", bufs=3) as rtp,
                tc.tile_pool(name="ps_a", bufs=2, space="PSUM") as psa,
                tc.tile_pool(name="ps_sw", bufs=1, space="PSUM") as psw,
                tc.tile_pool(name="ps_s0", bufs=1, space="PSUM") as spA,
            ):
                w_sb = xwpool.tile([128, 3 * KE * 128], BF16)
                nc.sync.dma_start(w_sb[:, 0:1024], wp[:, 0:1024])
                b_sb = xwpool.tile([128, 3], F32)
                nc.scalar.dma_start(b_sb[:], bqkv)
                x_sb = xwpool.tile([128, KE * LP], BF16)
                cos_sb = xwpool.tile([128, LP], BF16)
                sin_sb = xwpool.tile([128, LP], BF16)
                for bi, (n0, nw) in enumerate(NBLK):
                    nc.sync.dma_start(
                        x_sb[:, XOFF[bi]:XOFF[bi] + KE * nw],
                        xp[:, XOFF[bi]:XOFF[bi] + KE * nw],
                    )
                    if bi == 0:
                        nc.sync.dma_start(w_sb[:, 1024:3072], wp[:, 1024:3072])
                        nc.scalar.dma_start(cos_sb[:], cosT)
                        nc.scalar.dma_start(sin_sb[:], sinT)

                # warm the PE during the input-DMA wait so the first QKV
                # chunks run at full clock (HAM promotes after ~3.4us busy)
                warmup = psw.tile([128, 512], F32, tag="swp", name="warmup")
                for _ in range(30):
                    nc.tensor.matmul(warmup[:, 0:128], identb[:], identb[:])

                # mask columns of v_aug depend only on the mask DMA
                mview = mask_sb[:].rearrange("p (t o) -> p t o", o=1)
                nc.vector.tensor_copy(vaug[:, :, 64:65], mview)
                nc.vector.tensor_copy(vaug[:, :, 129:130], mview)

                def rope_chunk(T, n0, nw):
                    # rotate T[:, n0:n0+nw] in place; the 32-half swap within
                    # each head is a permutation matmul on PE.
                    swp = psw.tile([128, 512], F32, tag="swp", name=f"swp_{T.name}_{n0}")
                    nc.tensor.matmul(swp[:, :nw], perm_sb[:], T[:, n0:n0 + nw])
                    sw = rtp.tile([128, 512], BF16, tag="swap", name=f"sw_{T.name}_{n0}")
                    tmp = rtp.tile([128, 512], BF16, tag="tmp", name=f"tmp_{T.name}_{n0}")
                    nc.vector.tensor_mul(tmp[:, :nw], T[:, n0:n0 + nw], cos_sb[:, n0:n0 + nw])
                    nc.vector.tensor_mul(sw[:, :nw], swp[:, :nw], sin_sb[:, n0:n0 + nw])
                    nc.vector.tensor_add(T[:, n0:n0 + nw], tmp[:, :nw], sw[:, :nw])

                def vaug_chunk(n0, nw):
                    for t in range(n0 // 128, (n0 + nw) // 128):
                        tp = tpp.tile([128, 128], BF16, tag="tp", name="vtp")
                        nc.tensor.transpose(tp[:], V[:, 128 * t:128 * (t + 1)], identb[:])
                        nc.vector.tensor_scalar_mul(
                            vaug[:, t, 0:64], tp[:, 0:64], mask_sb[:, t:t + 1]
                        )
                        nc.vector.tensor_scalar_mul(
                            vaug[:, t, 65:129], tp[:, 64:128], mask_sb[:, t:t + 1]
                        )

                def feed(n):
                    while n and sched:
                        st_, t_ = sched.popleft()
                        if st_ is blk0:
                            score_t(st_, t_, spA)
                        else:
                            # deferred block-1 work: dedicated PSb tag so the
                            # long-lived pending tiles never share slots with
                            # block-0's stream (their reads are emitted later)
                            score_t(st_, t_, spA, pop=False, psb_tag="psb1")
                        n -= 1

                outs = [Q, K, V]
                for bi, (n0, nw) in enumerate(NBLK):
                    # newly-available score work given K roped through chunk bi-1
                    if bi >= 2:
                        kprog = n0 // 128
                        for t in range(hi[0], kprog):
                            sched.append((blk0, t))
                        hi[0] = kprog
                        if bi >= 3:  # Q[512:1024] roped after chunk 2
                            cap = min(kprog, BLK1_PHASE_A_CAP)
                            for t in range(hi[1], cap):
                                sched.append((blk1, t))
                            hi[1] = cap
                    for m in range(3):
                        feed(2)
                        ps = psa.tile([128, 512], F32, tag="qkvps")
                        for k in range(KE):
                            nc.tensor.matmul(
                                ps[:, :nw],
                                w_sb[:, 1024 * m + 128 * k:1024 * m + 128 * (k + 1)],
                                x_sb[:, XOFF[bi] + k * nw:XOFF[bi] + (k + 1) * nw],
                                start=(k == 0),
                                stop=(k == KE - 1),
                            )
                        nc.vector.tensor_scalar_add(
                            outs[m][:, n0:n0 + nw], ps[:, :nw], b_sb[:, m:m + 1]
                        )
                        if m < 2:
                            rope_chunk(outs[m], n0, nw)
                        else:
                            vaug_chunk(n0, nw)
                # flush remaining block-0 score work (K now fully roped)
                for t in range(hi[0], NK):
                    sched.append((blk0, t))
                hi[0] = NK
                feed(len(sched))

            # ---------------- Phase B: attention + AllToAll + projection ------
            with tc.tile_pool(name="pw_ag", bufs=1) as pwpool:
                # proj weights load during phase B so DMA is off the tail
                pw_sb = pwpool.tile([128, KE * E], BF16)
                nc.sync.dma_start(pw_sb[:], pwp)
                osb = pwpool.tile([128, LP], F32)

                def proj_job(pso, g, mE):
                    g0, w = GSPEC[g]
                    po = pso.tile([128, 512], F32, tag="po", name=f"po{g}_{mE}")
                    for k in range(KE):
                        nc.tensor.matmul(
                            po[:, 0:w],
                            pw_sb[:, 1024 * k + 128 * mE:1024 * k + 128 * (mE + 1)],
                            ag[g][:, k, :],
                            start=(k == 0),
                            stop=(k == KE - 1),
                        )
                    nc.vector.tensor_scalar_add(
                        osb[:, g0 + mE * w:g0 + (mE + 1) * w],
                        po[:, 0:w], pbias[:, mE:mE + 1],
                    )

                def evict_group(g):
                    g0, w = GSPEC[g]
                    nc.sync.dma_start(
                        outT[:, g0:g0 + 8 * w], osb[:, g0:g0 + 8 * w]
                    )

                with tc.tile_pool(name="ps_s", bufs=2, space="PSUM") as pss:
                    prev = blk0
                    for bi in range(1, 5):
                        st = blk1 if bi == 1 else BlockState(*LQB[bi])
                        t0 = hi[1] if bi == 1 else 0
                        lead = min(4, NK - t0)
                        # pop=False: ctx matmuls into the shared ctxf slot must
                        # not be emitted before finish_block(prev) reads it
                        for t in range(t0, t0 + lead):
                            score_t(st, t, pss, pop=False)
                        finish_block(prev, next_st=st)
                        if bi >= 2:
                            fire_group(bi - 2, pwpool)

                        for t in range(t0 + lead, NK):
                            score_t(st, t, pss)
                        prev = st

                    # ---- tail: last block norm + final small AllToAll ----
                    finish_block(prev)

                # projection: groups 0-2 overlap the final AllToAll transfer
                with tc.tile_pool(name="ps_o", bufs=4, space="PSUM") as pso:
                    fire_group(3, pwpool, tail=True)
                    for g in range(NG):
                        for mE in range(KE):
                            proj_job(pso, g, mE)
                        evict_group(g)

    nc.compile()
    _NC_CACHE["nc"] = nc
    return nc


def _prep_inputs(x, key_padding_mask, qkv_w, qkv_b, proj_w, proj_b, freqs_cos, freqs_sin):
    bf = ml_dtypes.bfloat16
    x = np.ascontiguousarray(np.asarray(x, np.float32))
    qkv_w = np.asarray(qkv_w, np.float32)
    qkv_b = np.asarray(qkv_b, np.float32)
    proj_w = np.asarray(proj_w, np.float32)
    proj_b = np.asarray(proj_b, np.float32)
    fc = np.asarray(freqs_cos, np.float32)  # [2304, 64]
    fs = np.asarray(freqs_sin, np.float32)
    mask = np.asarray(key_padding_mask)

    xpad = np.zeros((LP, E), np.float32)
    xpad[:L] = x
    xk = np.ascontiguousarray(xpad.T).reshape(KE, 128, LP)  # [k, p, n]
    xp = np.concatenate(
        [xk[:, :, n0:n0 + nw].transpose(1, 0, 2).reshape(128, KE * nw)
         for (n0, nw) in NBLK],
        axis=1,
    ).astype(bf)  # [128, KE*LP] chunk-major

    cosT = np.ones((64, LP), np.float32)
    cosT[:, 8:L] = fc.T
    cos2 = np.concatenate([cosT, cosT], axis=0).astype(bf)  # [128, LP]

    sinT = np.zeros((64, LP), np.float32)
    sinT[:, 8:L] = fs.T
    sinT[:32, :] *= -1.0  # sign of -x2 half folded into sin table
    sin2 = np.concatenate([sinT, sinT], axis=0).astype(bf)

    maskf = np.zeros((LP,), np.float32)
    maskf[:L] = mask.astype(np.float32)
    mskT = np.ascontiguousarray(maskf.reshape(NK, 128).T)  # [128, NK]

    pwT = np.ascontiguousarray(proj_w.T)  # [d, e]
    pwp = pwT.reshape(KE, 128, E).transpose(1, 0, 2).reshape(128, KE * E).astype(bf)
    pwp = np.ascontiguousarray(pwp)
    permM = np.zeros((128, 128), np.float32)  # lhsT: permM[k, m]=1 iff k==swap(m)
    for m128 in range(128):
        swp = m128 + 32 if (m128 % 64) < 32 else m128 - 32
        permM[swp, m128] = 1.0
    permM = permM.astype(bf)
    pb2 = np.ascontiguousarray(proj_b.reshape(KE, 128).T)  # [128, KE]

    in_maps = []
    for c in range(N_CORES):
        h0, h1 = 2 * c, 2 * c + 1
        rows = []
        bias_rows = []
        for sec in range(3):  # q, k, v sections of qkv_w
            for h in (h0, h1):
                sl = slice(1024 * sec + 64 * h, 1024 * sec + 64 * h + 64)
                rows.append(qkv_w[sl])
                bias_rows.append(qkv_b[sl])
        Wc = np.concatenate(rows, axis=0)           # [384, 1024]
        bc = np.concatenate(bias_rows, axis=0)      # [384]
        # m-major layout: wp[p, 1024*m + 128*k + c] = Wc[128m+c, 128k+p]
        wpc = (
            Wc.T.reshape(KE, 128, 3, 128)
            .transpose(1, 2, 0, 3)
            .reshape(128, 3 * KE * 128)
        )
        in_maps.append({
            "xp": xp,
            "wp": np.ascontiguousarray(wpc).astype(bf),
            "bqkv": np.ascontiguousarray(bc.reshape(3, 128).T),
            "cosT": cos2,
            "sinT": sin2,
            "mskT": mskT,
            "pwp": pwp,
            "pb": pb2,
            "perm": permM,
        })
    return in_maps


def _run(in_maps, trace=False):
    nc = _build()
    return run_bass_kernel_spmd(
        nc, in_maps, core_ids=list(range(N_CORES)), trace=trace
    )


def kernel(x, key_padding_mask, qkv_w, qkv_b, proj_w, proj_b, freqs_cos, freqs_sin):
    in_maps = _prep_inputs(
        x, key_padding_mask, qkv_w, qkv_b, proj_w, proj_b, freqs_cos, freqs_sin
    )
    res = _run(in_maps, trace=False)
    full = np.zeros((LP, E), np.float32)
    for c in range(N_CORES):
        r = np.asarray(res.results[c]["outT"])  # [128, LP]
        for g, (g0, w) in enumerate(GSPEC):
            block = r[:, g0:g0 + 8 * w].reshape(128, KE, w)   # [p, mE, n]
            full[g0 + w * c:g0 + w * (c + 1), :] = (
                block.transpose(2, 1, 0).reshape(w, E)
            )
    return np.ascontiguousarray(full[:L]).astype(np.float32)


# revision 21
# speedup vs baseline: 1.1866x; 1.0331x over previous
"""Multi-head attention (16 heads, L=2312, E=1024) on 8 trn2 NeuronCores.

Sharding: tensor-parallel over heads — each core computes 2 heads' full
attention (QKV proj + RoPE + softmax(QK^T)V), then 4 pipelined AllToAlls
re-shard context from head-split to interleaved sequence blocks so each
core computes a disjoint column set of the output projection while later
attention blocks are still in flight. Host reassembles the interleaved
blocks.

Key structure vs a naive port:
 - score matmuls contract over d=64 per head; the two heads live in
   disjoint SBUF partition halves, so the two matmuls land on disjoint
   PE row-groups and run concurrently (array packing).
 - context matmul is "flipped": exp-scores are the stationary operand
   (per 128-query subtile) and V^T (+ mask/ones column) streams with
   N=65, which both halves the streamed columns and yields the softmax
   denominator in the free dimension — normalization becomes lane-local
   vector work followed by a single 128x128 transpose per query tile.
 - block-0 (and early block-1) score/exp work is threaded between the
   QKV chunk matmuls so the ScalarE exp stream starts early; the output
   projection for already-arrived AllToAll groups is threaded into the
   later attention blocks so the PE never idles (keeps the HAM clock up)
   and the tail only carries the last 384 columns.
 - all DRAM inputs are laid out host-side so every DMA is 128
   contiguous rows (fast descriptor issue).

Numerics: bf16 operands with fp32 PSUM accumulation + fp32 softmax.

Self-contained: all shapes hardcoded; takes full unsharded inputs.
"""
from collections import deque

import numpy as np
import ml_dtypes

import concourse.bacc as bacc
import concourse.tile as tile
from concourse import mybir
from concourse.bass_utils import run_bass_kernel_spmd
from concourse.masks import make_identity

N_CORES = 8
L = 2312           # valid sequence length
LP = 2432          # padded to 19*128
NK = LP // 128     # 19 key tiles
E = 1024
KE = E // 128      # 8 contraction tiles over embed dim
F32 = mybir.dt.float32
BF16 = mybir.dt.bfloat16
I32 = mybir.dt.int32
SCALE = 0.125      # 1/sqrt(64)
# Schraudolph fast-exp constants (scale folded in); used on the DVE for a
# slice of the score columns to take load off the ScalarE exp stream
FE_A = float((1 << 23) * 1.4426950408889634 * SCALE)
FE_B = float(127.0 * (1 << 23) - 366392.3)
FE_C = 320        # columns [1024-FE_C : 1024) computed on the DVE

# lq blocks: (start, width); widths multiples of 128 except last (2312-2048=264)
LQB = [(0, 512), (512, 512), (1024, 512), (1536, 512), (2048, 264)]
# AllToAll groups (col_start, per-core width); group g becomes available
# after block g+1 is normalized. The last group is small to shrink the tail.
GSPEC = [(0, 84), (672, 86), (1360, 86), (2048, 48)]
NG = len(GSPEC)
# qkv N blocks over padded seq
NBLK = [(0, 256), (256, 256), (512, 512), (1024, 512), (1536, 512), (2048, 384)]
XOFF = []
_o = 0
for (_n0, _nw) in NBLK:
    XOFF.append(_o)
    _o += KE * _nw

_NC_CACHE = {}


def _subtiles(lqw):
    """(offset-in-block, width, ctx-tile-id, col-offset) per 128-query subtile."""
    out = []
    s = 0
    off = 0
    while off < lqw:
        sw = min(128, lqw - off)
        if s < 3:
            out.append((off, sw, 0, 130 * s))
        else:
            out.append((off, sw, 1, 0))
        s += 1
        off += sw
    return out


def _build():
    if "nc" in _NC_CACHE:
        return _NC_CACHE["nc"]
    nc = bacc.Bacc(
        "TRN2",
        target_bir_lowering=False,
        debug=False,
        enable_asserts=False,
        num_devices=N_CORES,
    )
    xp = nc.dram_tensor("xp", [128, KE * LP], BF16, kind="ExternalInput").ap()
    wp = nc.dram_tensor("wp", [128, 3 * KE * 128], BF16, kind="ExternalInput").ap()
    bqkv = nc.dram_tensor("bqkv", [128, 3], F32, kind="ExternalInput").ap()
    cosT = nc.dram_tensor("cosT", [128, LP], BF16, kind="ExternalInput").ap()
    sinT = nc.dram_tensor("sinT", [128, LP], BF16, kind="ExternalInput").ap()
    mskT = nc.dram_tensor("mskT", [128, NK], F32, kind="ExternalInput").ap()
    pwp = nc.dram_tensor("pwp", [128, KE * E], BF16, kind="ExternalInput").ap()
    pb = nc.dram_tensor("pb", [128, KE], F32, kind="ExternalInput").ap()
    perm = nc.dram_tensor("perm", [128, 128], BF16, kind="ExternalInput").ap()
    outT = nc.dram_tensor("outT", [128, LP], F32, kind="ExternalOutput").ap()

    with tile.TileContext(nc) as tc:
        with (
            tc.tile_pool(name="const", bufs=1) as cpool,
            tc.tile_pool(name="dram", bufs=1, space="DRAM") as dpool,
            tc.tile_pool(name="qkv", bufs=1) as qkvpool,
            tc.tile_pool(name="vaugp", bufs=1) as vaugpool,
            tc.tile_pool(name="ctxp", bufs=1) as ctxpool,
            tc.tile_pool(name="psb", bufs=12) as pspool,
            tc.tile_pool(name="cn", bufs=2) as cnpool,
            tc.tile_pool(name="rp", bufs=2) as rpool,
            tc.tile_pool(name="ps_c", bufs=1, space="PSUM") as psc,
            tc.tile_pool(name="ps_tp", bufs=1, space="PSUM") as tpp,
        ):
            identb = cpool.tile([128, 128], BF16)
            make_identity(nc, identb[:])
            pbias = cpool.tile([128, KE], F32)
            nc.gpsimd.dma_start(pbias[:], pb)
            mask_sb = cpool.tile([128, NK], F32)
            nc.gpsimd.dma_start(mask_sb[:], mskT)
            perm_sb = cpool.tile([128, 128], BF16)
            nc.gpsimd.dma_start(perm_sb[:], perm)

            Q = qkvpool.tile([128, LP], BF16)
            K = qkvpool.tile([128, LP], BF16)
            V = qkvpool.tile([128, LP], BF16)
            vaug = vaugpool.tile([128, NK, 130], BF16)
            ctxTn = ctxpool.tile([128, LP], BF16)
            cc_in = [
                dpool.tile([N_CORES, 128, w], BF16, name=f"cc_in{g}")
                for g, (_, w) in enumerate(GSPEC)
            ]
            cc_out = [
                dpool.tile([N_CORES, 128, w], BF16, name=f"cc_out{g}")
                for g, (_, w) in enumerate(GSPEC)
            ]

            # --- per-block attention state -------------------------------
            class BlockState:
                def __init__(self, lq0, lqw):
                    self.lq0, self.lqw = lq0, lqw
                    self.subs = _subtiles(lqw)
                    self.ctx = [
                        psc.tile([128, 390], F32, tag="ctxfA", name=f"ctxfA_{lq0}"),
                    ]
                    if any(ti == 1 for (_, _, ti, _) in self.subs):
                        self.ctx.append(
                            psc.tile([128, 130], F32, tag="ctxfB", name=f"ctxfB_{lq0}")
                        )
                    self.pending = []

                def init_ctx(self):
                    # PE 'start' clears has_written at bank granularity, so
                    # multiple accumulation groups per bank must be seeded by
                    # a vector memset and accumulate with start=False.
                    for ctile in self.ctx:
                        nc.vector.memset(ctile[:], 0.0)

            def ctx_mms(st, t, PSb):
                for (soff, sw, ti, coff) in st.subs:
                    ctile = st.ctx[ti]
                    for h in range(2):
                        nc.tensor.matmul(
                            ctile[0:sw, coff + 65 * h:coff + 65 * h + 65],
                            PSb[:, 512 * h + soff:512 * h + soff + sw],
                            vaug[:, t, 65 * h:65 * h + 65],
                            start=False,
                            stop=(t == NK - 1),
                        )

            def score_t(st, t, sp_pool, pop=True, psb_tag="psb", offload=False):
                SP = sp_pool.tile([128, 1024], F32, tag="sp", name="sp")
                lq0, lqw = st.lq0, st.lqw
                for h in range(2):
                    nc.tensor.matmul(
                        SP[:, 512 * h:512 * h + lqw],
                        K[64 * h:64 * h + 64, 128 * t:128 * (t + 1)],
                        Q[64 * h:64 * h + 64, lq0:lq0 + lqw],
                    )
                PSb = pspool.tile(
                    [128, 1024], BF16, tag=psb_tag, name="psb",
                    bufs=14 if psb_tag == "psb1" else None,
                )
                if lqw == 512 and offload:
                    nc.scalar.activation(
                        PSb[:, 0:1024 - FE_C], SP[:, 0:1024 - FE_C],
                        mybir.ActivationFunctionType.Exp, scale=SCALE,
                    )
                    ti = pspool.tile(
                        [128, FE_C], I32, tag="fei", name="fei", bufs=3
                    )
                    nc.vector.tensor_scalar(
                        out=ti[:], in0=SP[:, 1024 - FE_C:1024],
                        scalar1=FE_A, scalar2=FE_B,
                        op0=mybir.AluOpType.mult, op1=mybir.AluOpType.add,
                    )
                    nc.vector.tensor_copy(
                        PSb[:, 1024 - FE_C:1024], ti[:].bitcast(F32)
                    )
                elif lqw == 512:
                    nc.scalar.activation(
                        PSb[:], SP[:],
                        mybir.ActivationFunctionType.Exp, scale=SCALE,
                    )
                else:
                    for h in range(2):
                        nc.scalar.activation(
                            PSb[:, 512 * h:512 * h + lqw],
                            SP[:, 512 * h:512 * h + lqw],
                            mybir.ActivationFunctionType.Exp, scale=SCALE,
                        )
                st.pending.append((t, PSb))
                if pop:
                    # drain deferred work gradually (max 2/call) so ctx bursts
                    # never queue the next scores (and exp) out more than ~1 t
                    pops = 0
                    while len(st.pending) >= 2 and pops < 2:
                        ctx_mms(st, *st.pending.pop(0))
                        pops += 1

            def finish_block(st, next_st=None):
                for tp_, pb_ in st.pending:
                    ctx_mms(st, tp_, pb_)
                st.pending.clear()
                cnfs = []
                for (soff, sw, ti, coff) in st.subs:
                    ctile = st.ctx[ti]
                    CNF = cnpool.tile([128, 128], BF16, tag="cnf", name="cnf")
                    for h in range(2):
                        Rc = rpool.tile([128, 1], F32, tag="rc", name="rc")
                        nc.vector.reciprocal(
                            Rc[0:sw, :],
                            ctile[0:sw, coff + 65 * h + 64:coff + 65 * h + 65],
                        )
                        nc.vector.tensor_scalar_mul(
                            CNF[0:sw, 64 * h:64 * h + 64],
                            ctile[0:sw, coff + 65 * h:coff + 65 * h + 64],
                            Rc[0:sw, :],
                        )
                    cnfs.append(CNF)
                if next_st is not None:
                    # all ctile reads are emitted; clear the slot for the next
                    # block as early as possible so its ctx matmuls can start
                    next_st.init_ctx()
                for (soff, sw, ti, coff), CNF in zip(st.subs, cnfs):
                    TP = tpp.tile([128, 128], BF16, tag="tp", name="tp")
                    nc.tensor.transpose(
                        TP[:, 0:sw], CNF[0:sw, :], identb[0:sw, 0:sw]
                    )
                    nc.vector.tensor_copy(
                        ctxTn[:, st.lq0 + soff:st.lq0 + soff + sw], TP[:, 0:sw]
                    )

            ag = [None] * NG

            def fire_group(g, pwpool, tail=False):
                # stage ctxTn -> cc_in[g], AllToAll, unstage into SBUF
                g0, w = GSPEC[g]
                engines = (
                    [nc.gpsimd, nc.sync, nc.scalar] if tail
                    else [nc.gpsimd, nc.sync]
                )
                for j in range(N_CORES):
                    engines[j % len(engines)].dma_start(
                        cc_in[g][j], ctxTn[:, g0 + w * j:g0 + w * (j + 1)]
                    )
                nc.gpsimd.collective_compute(
                    "AllToAll",
                    mybir.AluOpType.bypass,
                    replica_groups=[list(range(N_CORES))],
                    ins=[cc_in[g].opt()],
                    outs=[cc_out[g].opt()],
                )
                ag[g] = pwpool.tile([128, KE, w], BF16, tag=f"ag{g}", name=f"ag{g}")
                for j in range(N_CORES):
                    engines[j % len(engines)].dma_start(ag[g][:, j, :], cc_out[g][j])

            # ---------------- Phase A: QKV projection + RoPE + V transpose ----
            blk0 = BlockState(*LQB[0])
            blk0.init_ctx()
            blk1 = BlockState(*LQB[1])
            sched = deque()   # (state, t) score work to thread into the chunks
            hi = {0: 0, 1: 0}
            BLK1_PHASE_A_CAP = 12
            with (
                tc.tile_pool(name="xw", bufs=1) as xwpool,
                tc.tile_pool(name="ropet", bufs=3) as rtp,
                tc.tile_pool(name="ps_a", bufs=2, space="PSUM") as psa,
                tc.tile_pool(name="ps_sw", bufs=1, space="PSUM") as psw,
                tc.tile_pool(name="ps_s0", bufs=1, space="PSUM") as spA,
            ):
                w_sb = xwpool.tile([128, 3 * KE * 128], BF16)
                nc.sync.dma_start(w_sb[:, 0:1024], wp[:, 0:1024])
                b_sb = xwpool.tile([128, 3], F32)
                nc.scalar.dma_start(b_sb[:], bqkv)
                x_sb = xwpool.tile([128, KE * LP], BF16)
                cos_sb = xwpool.tile([128, LP], BF16)
                sin_sb = xwpool.tile([128, LP], BF16)
                for bi, (n0, nw) in enumerate(NBLK):
                    nc.sync.dma_start(
                        x_sb[:, XOFF[bi]:XOFF[bi] + KE * nw],
                        xp[:, XOFF[bi]:XOFF[bi] + KE * nw],
                    )
                    if bi == 0:
                        nc.sync.dma_start(w_sb[:, 1024:3072], wp[:, 1024:3072])
                        nc.scalar.dma_start(cos_sb[:], cosT)
                        nc.scalar.dma_start(sin_sb[:], sinT)

                # warm the PE during the input-DMA wait so the first QKV
                # chunks run at full clock (HAM promotes after ~3.4us busy)
                warmup = psw.tile([128, 512], F32, tag="swp", name="warmup")
                for _ in range(30):
                    nc.tensor.matmul(warmup[:, 0:128], identb[:], identb[:])

                # mask columns of v_aug depend only on the mask DMA
                mview = mask_sb[:].rearrange("p (t o) -> p t o", o=1)
                nc.vector.tensor_copy(vaug[:, :, 64:65], mview)
                nc.vector.tensor_copy(vaug[:, :, 129:130], mview)

                def rope_chunk(T, n0, nw):
                    # rotate T[:, n0:n0+nw] in place; the 32-half swap within
                    # each head is a permutation matmul on PE.
                    swp = psw.tile([128, 512], F32, tag="swp", name=f"swp_{T.name}_{n0}")
                    nc.tensor.matmul(swp[:, :nw], perm_sb[:], T[:, n0:n0 + nw])
                    sw = rtp.tile([128, 512], BF16, tag="swap", name=f"sw_{T.name}_{n0}")
                    tmp = rtp.tile([128, 512], BF16, tag="tmp", name=f"tmp_{T.name}_{n0}")
                    nc.vector.tensor_mul(tmp[:, :nw], T[:, n0:n0 + nw], cos_sb[:, n0:n0 + nw])
                    nc.vector.tensor_mul(sw[:, :nw], swp[:, :nw], sin_sb[:, n0:n0 + nw])
                    nc.vector.tensor_add(T[:, n0:n0 + nw], tmp[:, :nw], sw[:, :nw])

                def vaug_chunk(n0, nw):
                    for t in range(n0 // 128, (n0 + nw) // 128):
                        tp = tpp.tile([128, 128], BF16, tag="tp", name="vtp")
                        nc.tensor.transpose(tp[:], V[:, 128 * t:128 * (t + 1)], identb[:])
                        nc.vector.tensor_scalar_mul(
                            vaug[:, t, 0:64], tp[:, 0:64], mask_sb[:, t:t + 1]
                        )
                        nc.vector.tensor_scalar_mul(
                            vaug[:, t, 65:129], tp[:, 64:128], mask_sb[:, t:t + 1]
                        )

                def feed(n):
                    while n and sched:
                        st_, t_ = sched.popleft()
                        if st_ is blk0:
                            score_t(st_, t_, spA)
                        else:
                            # deferred block-1 work: dedicated PSb tag so the
                            # long-lived pending tiles never share slots with
                            # block-0's stream (their reads are emitted later)
                            score_t(st_, t_, spA, pop=False, psb_tag="psb1")
                        n -= 1

                outs = [Q, K, V]
                for bi, (n0, nw) in enumerate(NBLK):
                    # newly-available score work given K roped through chunk bi-1
                    if bi >= 2:
                        kprog = n0 // 128
                        for t in range(hi[0], kprog):
                            sched.append((blk0, t))
                        hi[0] = kprog
                        if bi >= 3:  # Q[512:1024] roped after chunk 2
                            cap = min(kprog, BLK1_PHASE_A_CAP)
                            for t in range(hi[1], cap):
                                sched.append((blk1, t))
                            hi[1] = cap
                    for m in range(3):
                        feed(2)
                        ps = psa.tile([128, 512], F32, tag="qkvps")
                        for k in range(KE):
                            nc.tensor.matmul(
                                ps[:, :nw],
                                w_sb[:, 1024 * m + 128 * k:1024 * m + 128 * (k + 1)],
                                x_sb[:, XOFF[bi] + k * nw:XOFF[bi] + (k + 1) * nw],
                                start=(k == 0),
                                stop=(k == KE - 1),
                            )
                        nc.vector.tensor_scalar_add(
                            outs[m][:, n0:n0 + nw], ps[:, :nw], b_sb[:, m:m + 1]
                        )
                        if m < 2:
                            rope_chunk(outs[m], n0, nw)
                        else:
                            vaug_chunk(n0, nw)
                # flush remaining block-0 score work (K now fully roped)
                for t in range(hi[0], NK):
                    sched.append((blk0, t))
                hi[0] = NK
                feed(len(sched))

            # ---------------- Phase B: attention + AllToAll + projection ------
            with tc.tile_pool(name="pw_ag", bufs=1) as pwpool:
                # proj weights load during phase B so DMA is off the tail
                pw_sb = pwpool.tile([128, KE * E], BF16)
                nc.sync.dma_start(pw_sb[:], pwp)
                osb = pwpool.tile([128, LP], F32)

                def proj_job(pso, g, mE):
                    g0, w = GSPEC[g]
                    po = pso.tile([128, 512], F32, tag="po", name=f"po{g}_{mE}")
                    for k in range(KE):
                        nc.tensor.matmul(
                            po[:, 0:w],
                            pw_sb[:, 1024 * k + 128 * mE:1024 * k + 128 * (mE + 1)],
                            ag[g][:, k, :],
                            start=(k == 0),
                            stop=(k == KE - 1),
                        )
                    nc.vector.tensor_scalar_add(
                        osb[:, g0 + mE * w:g0 + (mE + 1) * w],
                        po[:, 0:w], pbias[:, mE:mE + 1],
                    )

                def evict_group(g):
                    g0, w = GSPEC[g]
                    nc.sync.dma_start(
                        outT[:, g0:g0 + 8 * w], osb[:, g0:g0 + 8 * w]
                    )

                with tc.tile_pool(name="ps_s", bufs=2, space="PSUM") as pss:
                    prev = blk0
                    for bi in range(1, 5):
                        st = blk1 if bi == 1 else BlockState(*LQB[bi])
                        t0 = hi[1] if bi == 1 else 0
                        lead = min(4, NK - t0)
                        # pop=False: ctx matmuls into the shared ctxf slot must
                        # not be emitted before finish_block(prev) reads it
                        for t in range(t0, t0 + lead):
                            score_t(st, t, pss, pop=False)
                        finish_block(prev, next_st=st)
                        if bi >= 2:
                            fire_group(bi - 2, pwpool)

                        for t in range(t0 + lead, NK):
                            score_t(st, t, pss)
                        prev = st

                    # ---- tail: last block norm + final small AllToAll ----
                    finish_block(prev)

                # projection: groups 0-2 overlap the final AllToAll transfer
                with tc.tile_pool(name="ps_o", bufs=4, space="PSUM") as pso:
                    fire_group(3, pwpool, tail=True)
                    for g in range(NG):
                        for mE in range(KE):
                            proj_job(pso, g, mE)
                        evict_group(g)

    nc.compile()
    _NC_CACHE["nc"] = nc
    return nc


def _prep_inputs(x, key_padding_mask, qkv_w, qkv_b, proj_w, proj_b, freqs_cos, freqs_sin):
    bf = ml_dtypes.bfloat16
    x = np.ascontiguousarray(np.asarray(x, np.float32))
    qkv_w = np.asarray(qkv_w, np.float32)
    qkv_b = np.asarray(qkv_b, np.float32)
    proj_w = np.asarray(proj_w, np.float32)
    proj_b = np.asarray(proj_b, np.float32)
    fc = np.asarray(freqs_cos, np.float32)  # [2304, 64]
    fs = np.asarray(freqs_sin, np.float32)
    mask = np.asarray(key_padding_mask)

    xpad = np.zeros((LP, E), np.float32)
    xpad[:L] = x
    xk = np.ascontiguousarray(xpad.T).reshape(KE, 128, LP)  # [k, p, n]
    xp = np.concatenate(
        [xk[:, :, n0:n0 + nw].transpose(1, 0, 2).reshape(128, KE * nw)
         for (n0, nw) in NBLK],
        axis=1,
    ).astype(bf)  # [128, KE*LP] chunk-major

    cosT = np.ones((64, LP), np.float32)
    cosT[:, 8:L] = fc.T
    cos2 = np.concatenate([cosT, cosT], axis=0).astype(bf)  # [128, LP]

    sinT = np.zeros((64, LP), np.float32)
    sinT[:, 8:L] = fs.T
    sinT[:32, :] *= -1.0  # sign of -x2 half folded into sin table
    sin2 = np.concatenate([sinT, sinT], axis=0).astype(bf)

    maskf = np.zeros((LP,), np.float32)
    maskf[:L] = mask.astype(np.float32)
    mskT = np.ascontiguousarray(maskf.reshape(NK, 128).T)  # [128, NK]

    pwT = np.ascontiguousarray(proj_w.T)  # [d, e]
    pwp = pwT.reshape(KE, 128, E).transpose(1, 0, 2).reshape(128, KE * E).astype(bf)
    pwp = np.ascontiguousarray(pwp)
    permM = np.zeros((128, 128), np.float32)  # lhsT: permM[k, m]=1 iff k==swap(m)
    for m128 in range(128):
        swp = m128 + 32 if (m128 % 64) < 32 else m128 - 32
        permM[swp, m128] = 1.0
    permM = permM.astype(bf)
    pb2 = np.ascontiguousarray(proj_b.reshape(KE, 128).T)  # [128, KE]

    in_maps = []
    for c in range(N_CORES):
        h0, h1 = 2 * c, 2 * c + 1
        rows = []
        bias_rows = []
        for sec in range(3):  # q, k, v sections of qkv_w
            for h in (h0, h1):
                sl = slice(1024 * sec + 64 * h, 1024 * sec + 64 * h + 64)
                rows.append(qkv_w[sl])
                bias_rows.append(qkv_b[sl])
        Wc = np.concatenate(rows, axis=0)           # [384, 1024]
        bc = np.concatenate(bias_rows, axis=0)      # [384]
        # m-major layout: wp[p, 1024*m + 128*k + c] = Wc[128m+c, 128k+p]
        wpc = (
            Wc.T.reshape(KE, 128, 3, 128)
            .transpose(1, 2, 0, 3)
            .reshape(128, 3 * KE * 128)
        )
        in_maps.append({
            "xp": xp,
            "wp": np.ascontiguousarray(wpc).astype(bf),
            "bqkv": np.ascontiguousarray(bc.reshape(3, 128).T),
            "cosT": cos2,
            "sinT": sin2,
            "mskT": mskT,
            "pwp": pwp,
            "pb": pb2,
            "perm": permM,
        })
    return in_maps


def _run(in_maps, trace=False):
    nc = _build()
    return run_bass_kernel_spmd(
        nc, in_maps, core_ids=list(range(N_CORES)), trace=trace
    )


def kernel(x, key_padding_mask, qkv_w, qkv_b, proj_w, proj_b, freqs_cos, freqs_sin):
    in_maps = _prep_inputs(
        x, key_padding_mask, qkv_w, qkv_b, proj_w, proj_b, freqs_cos, freqs_sin
    )
    res = _run(in_maps, trace=False)
    full = np.zeros((LP, E), np.float32)
    for c in range(N_CORES):
        r = np.asarray(res.results[c]["outT"])  # [128, LP]
        for g, (g0, w) in enumerate(GSPEC):
            block = r[:, g0:g0 + 8 * w].reshape(128, KE, w)   # [p, mE, n]
            full[g0 + w * c:g0 + w * (c + 1), :] = (
                block.transpose(2, 1, 0).reshape(w, E)
            )
    return np.ascontiguousarray(full[:L]).astype(np.float32)


# revision 23
# speedup vs baseline: 1.2796x; 1.0784x over previous
"""Multi-head attention (16 heads, L=2312, E=1024) on 8 trn2 NeuronCores.

Sharding: tensor-parallel over heads — each core computes 2 heads' full
attention (QKV proj + RoPE + softmax(QK^T)V), then 4 pipelined AllToAlls
re-shard context from head-split to interleaved sequence blocks so each
core computes a disjoint column set of the output projection while later
attention blocks are still in flight. Host reassembles the interleaved
blocks.

Key structure vs a naive port:
 - score matmuls contract over d=64 per head; the two heads live in
   disjoint SBUF partition halves, so the two matmuls land on disjoint
   PE row-groups and run concurrently (array packing).
 - context matmul is "flipped": exp-scores are the stationary operand
   (per 128-query subtile) and V^T (+ mask/ones column) streams with
   N=65, which both halves the streamed columns and yields the softmax
   denominator in the free dimension — normalization becomes lane-local
   vector work followed by a single 128x128 transpose per query tile.
 - block-0 (and early block-1) score/exp work is threaded between the
   QKV chunk matmuls so the ScalarE exp stream starts early; the output
   projection for already-arrived AllToAll groups is threaded into the
   later attention blocks so the PE never idles (keeps the HAM clock up)
   and the tail only carries the last 384 columns.
 - all DRAM inputs are laid out host-side so every DMA is 128
   contiguous rows (fast descriptor issue).

Numerics: bf16 operands with fp32 PSUM accumulation + fp32 softmax.

Self-contained: all shapes hardcoded; takes full unsharded inputs.
"""
from collections import deque

import numpy as np
import ml_dtypes

import concourse.bacc as bacc
import concourse.tile as tile
from concourse import mybir
from concourse.bass_utils import run_bass_kernel_spmd
from concourse.masks import make_identity

N_CORES = 8
L = 2312           # valid sequence length
LP = 2432          # padded to 19*128
NK = LP // 128     # 19 key tiles
E = 1024
KE = E // 128      # 8 contraction tiles over embed dim
F32 = mybir.dt.float32
BF16 = mybir.dt.bfloat16
I32 = mybir.dt.int32
SCALE = 0.125      # 1/sqrt(64)
# Schraudolph fast-exp constants (scale folded in); used on the DVE for a
# slice of the score columns to take load off the ScalarE exp stream
FE_A = float((1 << 23) * 1.4426950408889634 * SCALE)
FE_B = float(127.0 * (1 << 23) - 366392.3)
FE_C = 320        # columns [1024-FE_C : 1024) computed on the DVE

# lq blocks: (start, width); widths multiples of 128 except last (2312-2048=264)
LQB = [(0, 512), (512, 512), (1024, 512), (1536, 512), (2048, 264)]
# AllToAll groups (col_start, per-core width); group g becomes available
# after block g+1 is normalized. The last group is small to shrink the tail.
GSPEC = [(0, 84), (672, 86), (1360, 86), (2048, 48)]
NG = len(GSPEC)
# qkv N blocks over padded seq
NBLK = [(0, 256), (256, 256), (512, 512), (1024, 512), (1536, 512), (2048, 384)]
XOFF = []
_o = 0
for (_n0, _nw) in NBLK:
    XOFF.append(_o)
    _o += KE * _nw

_NC_CACHE = {}


def _subtiles(lqw):
    """(offset-in-block, width, ctx-tile-id, col-offset) per 128-query subtile."""
    out = []
    s = 0
    off = 0
    while off < lqw:
        sw = min(128, lqw - off)
        if s < 3:
            out.append((off, sw, 0, 130 * s))
        else:
            out.append((off, sw, 1, 0))
        s += 1
        off += sw
    return out


def _build():
    if "nc" in _NC_CACHE:
        return _NC_CACHE["nc"]
    nc = bacc.Bacc(
        "TRN2",
        target_bir_lowering=False,
        debug=False,
        enable_asserts=False,
        num_devices=N_CORES,
    )
    xp = nc.dram_tensor("xp", [128, KE * LP], BF16, kind="ExternalInput").ap()
    wp = nc.dram_tensor("wp", [128, 3 * KE * 128], BF16, kind="ExternalInput").ap()
    bqkv = nc.dram_tensor("bqkv", [128, 3], F32, kind="ExternalInput").ap()
    cosT = nc.dram_tensor("cosT", [128, LP], BF16, kind="ExternalInput").ap()
    sinT = nc.dram_tensor("sinT", [128, LP], BF16, kind="ExternalInput").ap()
    mskT = nc.dram_tensor("mskT", [128, NK], F32, kind="ExternalInput").ap()
    pwp = nc.dram_tensor("pwp", [128, KE * E], BF16, kind="ExternalInput").ap()
    pb = nc.dram_tensor("pb", [128, KE], F32, kind="ExternalInput").ap()
    perm = nc.dram_tensor("perm", [128, 128], BF16, kind="ExternalInput").ap()
    outT = nc.dram_tensor("outT", [128, LP], F32, kind="ExternalOutput").ap()

    with tile.TileContext(nc) as tc:
        with (
            tc.tile_pool(name="const", bufs=1) as cpool,
            tc.tile_pool(name="dram", bufs=1, space="DRAM") as dpool,
            tc.tile_pool(name="qkv", bufs=1) as qkvpool,
            tc.tile_pool(name="vaugp", bufs=1) as vaugpool,
            tc.tile_pool(name="ctxp", bufs=1) as ctxpool,
            tc.tile_pool(name="psb", bufs=12) as pspool,
            tc.tile_pool(name="cn", bufs=2) as cnpool,
            tc.tile_pool(name="rp", bufs=2) as rpool,
            tc.tile_pool(name="ps_c", bufs=1, space="PSUM") as psc,
            tc.tile_pool(name="ps_tp", bufs=1, space="PSUM") as tpp,
        ):
            identb = cpool.tile([128, 128], BF16)
            make_identity(nc, identb[:])
            pbias = cpool.tile([128, KE], F32)
            nc.gpsimd.dma_start(pbias[:], pb)
            mask_sb = cpool.tile([128, NK], F32)
            nc.gpsimd.dma_start(mask_sb[:], mskT)
            perm_sb = cpool.tile([128, 128], BF16)
            nc.gpsimd.dma_start(perm_sb[:], perm)

            Q = qkvpool.tile([128, LP], BF16)
            K = qkvpool.tile([128, LP], BF16)
            V = qkvpool.tile([128, LP], BF16)
            vaug = vaugpool.tile([128, NK, 130], BF16)
            ctxTn = ctxpool.tile([128, LP], BF16)
            cc_in = [
                dpool.tile([N_CORES, 128, w], BF16, name=f"cc_in{g}")
                for g, (_, w) in enumerate(GSPEC)
            ]
            cc_out = [
                dpool.tile([N_CORES, 128, w], BF16, name=f"cc_out{g}")
                for g, (_, w) in enumerate(GSPEC)
            ]

            # --- per-block attention state -------------------------------
            class BlockState:
                def __init__(self, lq0, lqw):
                    self.lq0, self.lqw = lq0, lqw
                    self.hstr = 512
                    self.subs = _subtiles(lqw)
                    self.ctx = [
                        psc.tile([128, 390], F32, tag="ctxfA", name=f"ctxfA_{lq0}"),
                    ]
                    if any(ti == 1 for (_, _, ti, _) in self.subs):
                        self.ctx.append(
                            psc.tile([128, 130], F32, tag="ctxfB", name=f"ctxfB_{lq0}")
                        )
                    self.pending = []

                def init_ctx(self):
                    # PE 'start' clears has_written at bank granularity, so
                    # multiple accumulation groups per bank must be seeded by
                    # a vector memset and accumulate with start=False.
                    for ctile in self.ctx:
                        nc.vector.memset(ctile[:], 0.0)

            def ctx_mms(st, t, PSb):
                for (soff, sw, ti, coff) in st.subs:
                    ctile = st.ctx[ti]
                    for h in range(2):
                        nc.tensor.matmul(
                            ctile[0:sw, coff + 65 * h:coff + 65 * h + 65],
                            PSb[:, st.hstr * h + soff:st.hstr * h + soff + sw],
                            vaug[:, t, 65 * h:65 * h + 65],
                            start=False,
                            stop=(t == NK - 1),
                        )

            def score_t(st, t, sp_pool, pop=True, psb_tag="psb", offload=False):
                SP = sp_pool.tile([128, 1024], F32, tag="sp", name="sp")
                lq0, lqw = st.lq0, st.lqw
                hstr = st.hstr
                for h in range(2):
                    nc.tensor.matmul(
                        SP[:, hstr * h:hstr * h + lqw],
                        K[64 * h:64 * h + 64, 128 * t:128 * (t + 1)],
                        Q[64 * h:64 * h + 64, lq0:lq0 + lqw],
                    )
                PSb = pspool.tile(
                    [128, 1024], BF16, tag=psb_tag, name="psb",
                    bufs=14 if psb_tag == "psb1" else None,
                )
                if hstr * 2 == lqw * 2 or lqw == 512:
                    # both halves contiguous -> one exp instruction
                    nc.scalar.activation(
                        PSb[:, 0:2 * hstr], SP[:, 0:2 * hstr],
                        mybir.ActivationFunctionType.Exp, scale=SCALE,
                    )
                else:
                    for h in range(2):
                        nc.scalar.activation(
                            PSb[:, hstr * h:hstr * h + lqw],
                            SP[:, hstr * h:hstr * h + lqw],
                            mybir.ActivationFunctionType.Exp, scale=SCALE,
                        )
                st.pending.append((t, PSb))
                if pop:
                    # drain deferred work gradually (max 2/call) so ctx bursts
                    # never queue the next scores (and exp) out more than ~1 t
                    pops = 0
                    while len(st.pending) >= 2 and pops < 2:
                        ctx_mms(st, *st.pending.pop(0))
                        pops += 1

            def finish_block(st, next_st=None):
                for tp_, pb_ in st.pending:
                    ctx_mms(st, tp_, pb_)
                st.pending.clear()
                cnfs = []
                for (soff, sw, ti, coff) in st.subs:
                    ctile = st.ctx[ti]
                    CNF = cnpool.tile([128, 128], BF16, tag="cnf", name="cnf")
                    for h in range(2):
                        Rc = rpool.tile([128, 1], F32, tag="rc", name="rc")
                        nc.vector.reciprocal(
                            Rc[0:sw, :],
                            ctile[0:sw, coff + 65 * h + 64:coff + 65 * h + 65],
                        )
                        nc.vector.tensor_scalar_mul(
                            CNF[0:sw, 64 * h:64 * h + 64],
                            ctile[0:sw, coff + 65 * h:coff + 65 * h + 64],
                            Rc[0:sw, :],
                        )
                    cnfs.append(CNF)
                if next_st is not None:
                    # all ctile reads are emitted; clear the slot for the next
                    # block as early as possible so its ctx matmuls can start
                    next_st.init_ctx()
                for (soff, sw, ti, coff), CNF in zip(st.subs, cnfs):
                    TP = tpp.tile([128, 128], BF16, tag="tp", name="tp")
                    nc.tensor.transpose(
                        TP[:, 0:sw], CNF[0:sw, :], identb[0:sw, 0:sw]
                    )
                    nc.vector.tensor_copy(
                        ctxTn[:, st.lq0 + soff:st.lq0 + soff + sw], TP[:, 0:sw]
                    )

            ag = [None] * NG

            def fire_group(g, pwpool, tail=False):
                # stage ctxTn -> cc_in[g], AllToAll, unstage into SBUF
                g0, w = GSPEC[g]
                engines = (
                    [nc.gpsimd, nc.sync, nc.scalar] if tail
                    else [nc.gpsimd, nc.sync]
                )
                for j in range(N_CORES):
                    engines[j % len(engines)].dma_start(
                        cc_in[g][j], ctxTn[:, g0 + w * j:g0 + w * (j + 1)]
                    )
                nc.gpsimd.collective_compute(
                    "AllToAll",
                    mybir.AluOpType.bypass,
                    replica_groups=[list(range(N_CORES))],
                    ins=[cc_in[g].opt()],
                    outs=[cc_out[g].opt()],
                )
                ag[g] = pwpool.tile([128, KE, w], BF16, tag=f"ag{g}", name=f"ag{g}")
                for j in range(N_CORES):
                    engines[j % len(engines)].dma_start(ag[g][:, j, :], cc_out[g][j])

            # ---------------- Phase A: QKV projection + RoPE + V transpose ----
            blk0 = BlockState(*LQB[0])
            blk0.init_ctx()
            blk1 = BlockState(*LQB[1])
            sched = deque()   # (state, t) score work to thread into the chunks
            hi = {0: 0, 1: 0}
            BLK1_PHASE_A_CAP = 12
            with (
                tc.tile_pool(name="xw", bufs=1) as xwpool,
                tc.tile_pool(name="ropet", bufs=3) as rtp,
                tc.tile_pool(name="ps_a", bufs=2, space="PSUM") as psa,
                tc.tile_pool(name="ps_sw", bufs=1, space="PSUM") as psw,
                tc.tile_pool(name="ps_s0", bufs=1, space="PSUM") as spA,
            ):
                w_sb = xwpool.tile([128, 3 * KE * 128], BF16)
                nc.sync.dma_start(w_sb[:, 0:1024], wp[:, 0:1024])
                b_sb = xwpool.tile([128, 3], F32)
                nc.scalar.dma_start(b_sb[:], bqkv)
                x_sb = xwpool.tile([128, KE * LP], BF16)
                cos_sb = xwpool.tile([128, LP], BF16)
                sin_sb = xwpool.tile([128, LP], BF16)
                for bi, (n0, nw) in enumerate(NBLK):
                    nc.sync.dma_start(
                        x_sb[:, XOFF[bi]:XOFF[bi] + KE * nw],
                        xp[:, XOFF[bi]:XOFF[bi] + KE * nw],
                    )
                    if bi == 0:
                        nc.sync.dma_start(w_sb[:, 1024:3072], wp[:, 1024:3072])
                        nc.scalar.dma_start(cos_sb[:], cosT)
                        nc.scalar.dma_start(sin_sb[:], sinT)

                # warm the PE during the input-DMA wait so the first QKV
                # chunks run at full clock (HAM promotes after ~3.4us busy)
                warmup = psw.tile([128, 512], F32, tag="swp", name="warmup")
                for _ in range(30):
                    nc.tensor.matmul(warmup[:, 0:128], identb[:], identb[:])

                # mask columns of v_aug depend only on the mask DMA
                mview = mask_sb[:].rearrange("p (t o) -> p t o", o=1)
                nc.vector.tensor_copy(vaug[:, :, 64:65], mview)
                nc.vector.tensor_copy(vaug[:, :, 129:130], mview)

                def rope_chunk(T, n0, nw):
                    # rotate T[:, n0:n0+nw] in place; the 32-half swap within
                    # each head is a permutation matmul on PE.
                    swp = psw.tile([128, 512], F32, tag="swp", name=f"swp_{T.name}_{n0}")
                    nc.tensor.matmul(swp[:, :nw], perm_sb[:], T[:, n0:n0 + nw])
                    sw = rtp.tile([128, 512], BF16, tag="swap", name=f"sw_{T.name}_{n0}")
                    tmp = rtp.tile([128, 512], BF16, tag="tmp", name=f"tmp_{T.name}_{n0}")
                    nc.vector.tensor_mul(tmp[:, :nw], T[:, n0:n0 + nw], cos_sb[:, n0:n0 + nw])
                    nc.vector.tensor_mul(sw[:, :nw], swp[:, :nw], sin_sb[:, n0:n0 + nw])
                    nc.vector.tensor_add(T[:, n0:n0 + nw], tmp[:, :nw], sw[:, :nw])

                def vaug_chunk(n0, nw):
                    for t in range(n0 // 128, (n0 + nw) // 128):
                        tp = tpp.tile([128, 128], BF16, tag="tp", name="vtp")
                        nc.tensor.transpose(tp[:], V[:, 128 * t:128 * (t + 1)], identb[:])
                        nc.vector.tensor_scalar_mul(
                            vaug[:, t, 0:64], tp[:, 0:64], mask_sb[:, t:t + 1]
                        )
                        nc.vector.tensor_scalar_mul(
                            vaug[:, t, 65:129], tp[:, 64:128], mask_sb[:, t:t + 1]
                        )

                def feed(n):
                    while n and sched:
                        st_, t_ = sched.popleft()
                        if st_ is blk0:
                            score_t(st_, t_, spA)
                        else:
                            # deferred block-1 work: dedicated PSb tag so the
                            # long-lived pending tiles never share slots with
                            # block-0's stream (their reads are emitted later)
                            score_t(st_, t_, spA, pop=False, psb_tag="psb1")
                        n -= 1

                outs = [Q, K, V]
                for bi, (n0, nw) in enumerate(NBLK):
                    # newly-available score work given K roped through chunk bi-1
                    if bi >= 2:
                        kprog = n0 // 128
                        for t in range(hi[0], kprog):
                            sched.append((blk0, t))
                        hi[0] = kprog
                        if bi >= 3:  # Q[512:1024] roped after chunk 2
                            cap = min(kprog, BLK1_PHASE_A_CAP)
                            for t in range(hi[1], cap):
                                sched.append((blk1, t))
                            hi[1] = cap
                    for m in range(3):
                        feed(2)
                        ps = psa.tile([128, 512], F32, tag="qkvps")
                        for k in range(KE):
                            nc.tensor.matmul(
                                ps[:, :nw],
                                w_sb[:, 1024 * m + 128 * k:1024 * m + 128 * (k + 1)],
                                x_sb[:, XOFF[bi] + k * nw:XOFF[bi] + (k + 1) * nw],
                                start=(k == 0),
                                stop=(k == KE - 1),
                            )
                        nc.vector.tensor_scalar_add(
                            outs[m][:, n0:n0 + nw], ps[:, :nw], b_sb[:, m:m + 1]
                        )
                        if m < 2:
                            rope_chunk(outs[m], n0, nw)
                        else:
                            vaug_chunk(n0, nw)
                # flush remaining block-0 score work (K now fully roped)
                for t in range(hi[0], NK):
                    sched.append((blk0, t))
                hi[0] = NK
                feed(len(sched))

            # ---------------- Phase B: attention + AllToAll + projection ------
            with tc.tile_pool(name="pw_ag", bufs=1) as pwpool:
                # proj weights load during phase B so DMA is off the tail
                pw_sb = pwpool.tile([128, KE * E], BF16)
                nc.sync.dma_start(pw_sb[:], pwp)
                osb = pwpool.tile([128, LP], F32)

                def proj_job(pso, g, mE):
                    g0, w = GSPEC[g]
                    po = pso.tile([128, 512], F32, tag="po", name=f"po{g}_{mE}")
                    for k in range(KE):
                        nc.tensor.matmul(
                            po[:, 0:w],
                            pw_sb[:, 1024 * k + 128 * mE:1024 * k + 128 * (mE + 1)],
                            ag[g][:, k, :],
                            start=(k == 0),
                            stop=(k == KE - 1),
                        )
                    nc.vector.tensor_scalar_add(
                        osb[:, g0 + mE * w:g0 + (mE + 1) * w],
                        po[:, 0:w], pbias[:, mE:mE + 1],
                    )

                def evict_group(g):
                    g0, w = GSPEC[g]
                    nc.sync.dma_start(
                        outT[:, g0:g0 + 8 * w], osb[:, g0:g0 + 8 * w]
                    )

                with tc.tile_pool(name="ps_s", bufs=2, space="PSUM") as pss:
                    prev = blk0
                    for bi in range(1, 5):
                        st = blk1 if bi == 1 else BlockState(*LQB[bi])
                        t0 = hi[1] if bi == 1 else 0
                        lead = min(4, NK - t0)
                        # pop=False: ctx matmuls into the shared ctxf slot must
                        # not be emitted before finish_block(prev) reads it
                        for t in range(t0, t0 + lead):
                            score_t(st, t, pss, pop=False)
                        finish_block(prev, next_st=st)
                        if bi >= 2:
                            fire_group(bi - 2, pwpool)

                        for t in range(t0 + lead, NK):
                            score_t(st, t, pss)
                        prev = st

                    # ---- tail: last block norm + final small AllToAll ----
                    finish_block(prev)

                # projection: groups 0-2 overlap the final AllToAll transfer
                with tc.tile_pool(name="ps_o", bufs=4, space="PSUM") as pso:
                    fire_group(3, pwpool, tail=True)
                    for g in range(NG):
                        for mE in range(KE):
                            proj_job(pso, g, mE)
                        evict_group(g)

    nc.compile()
    _NC_CACHE["nc"] = nc
    return nc


def _prep_inputs(x, key_padding_mask, qkv_w, qkv_b, proj_w, proj_b, freqs_cos, freqs_sin):
    bf = ml_dtypes.bfloat16
    x = np.ascontiguousarray(np.asarray(x, np.float32))
    qkv_w = np.asarray(qkv_w, np.float32)
    qkv_b = np.asarray(qkv_b, np.float32)
    proj_w = np.asarray(proj_w, np.float32)
    proj_b = np.asarray(proj_b, np.float32)
    fc = np.asarray(freqs_cos, np.float32)  # [2304, 64]
    fs = np.asarray(freqs_sin, np.float32)
    mask = np.asarray(key_padding_mask)

    xpad = np.zeros((LP, E), np.float32)
    xpad[:L] = x
    xk = np.ascontiguousarray(xpad.T).reshape(KE, 128, LP)  # [k, p, n]
    xp = np.concatenate(
        [xk[:, :, n0:n0 + nw].transpose(1, 0, 2).reshape(128, KE * nw)
         for (n0, nw) in NBLK],
        axis=1,
    ).astype(bf)  # [128, KE*LP] chunk-major

    cosT = np.ones((64, LP), np.float32)
    cosT[:, 8:L] = fc.T
    cos2 = np.concatenate([cosT, cosT], axis=0).astype(bf)  # [128, LP]

    sinT = np.zeros((64, LP), np.float32)
    sinT[:, 8:L] = fs.T
    sinT[:32, :] *= -1.0  # sign of -x2 half folded into sin table
    sin2 = np.concatenate([sinT, sinT], axis=0).astype(bf)

    maskf = np.zeros((LP,), np.float32)
    maskf[:L] = mask.astype(np.float32)
    mskT = np.ascontiguousarray(maskf.reshape(NK, 128).T)  # [128, NK]

    pwT = np.ascontiguousarray(proj_w.T)  # [d, e]
    pwp = pwT.reshape(KE, 128, E).transpose(1, 0, 2).reshape(128, KE * E).astype(bf)
    pwp = np.ascontiguousarray(pwp)
    permM = np.zeros((128, 128), np.float32)  # lhsT: permM[k, m]=1 iff k==swap(m)
    for m128 in range(128):
        swp = m128 + 32 if (m128 % 64) < 32 else m128 - 32
        permM[swp, m128] = 1.0
    permM = permM.astype(bf)
    pb2 = np.ascontiguousarray(proj_b.reshape(KE, 128).T)  # [128, KE]

    in_maps = []
    for c in range(N_CORES):
        h0, h1 = 2 * c, 2 * c + 1
        rows = []
        bias_rows = []
        for sec in range(3):  # q, k, v sections of qkv_w
            for h in (h0, h1):
                sl = slice(1024 * sec + 64 * h, 1024 * sec + 64 * h + 64)
                rows.append(qkv_w[sl])
                bias_rows.append(qkv_b[sl])
        Wc = np.concatenate(rows, axis=0)           # [384, 1024]
        bc = np.concatenate(bias_rows, axis=0)      # [384]
        # m-major layout: wp[p, 1024*m + 128*k + c] = Wc[128m+c, 128k+p]
        wpc = (
            Wc.T.reshape(KE, 128, 3, 128)
            .transpose(1, 2, 0, 3)
            .reshape(128, 3 * KE * 128)
        )
        in_maps.append({
            "xp": xp,
            "wp": np.ascontiguousarray(wpc).astype(bf),
            "bqkv": np.ascontiguousarray(bc.reshape(3, 128).T),
            "cosT": cos2,
            "sinT": sin2,
            "mskT": mskT,
            "pwp": pwp,
            "pb": pb2,
            "perm": permM,
        })
    return in_maps


def _run(in_maps, trace=False):
    nc = _build()
    return run_bass_kernel_spmd(
        nc, in_maps, core_ids=list(range(N_CORES)), trace=trace
    )


def kernel(x, key_padding_mask, qkv_w, qkv_b, proj_w, proj_b, freqs_cos, freqs_sin):
    in_maps = _prep_inputs(
        x, key_padding_mask, qkv_w, qkv_b, proj_w, proj_b, freqs_cos, freqs_sin
    )
    res = _run(in_maps, trace=False)
    full = np.zeros((LP, E), np.float32)
    for c in range(N_CORES):
        r = np.asarray(res.results[c]["outT"])  # [128, LP]
        for g, (g0, w) in enumerate(GSPEC):
            block = r[:, g0:g0 + 8 * w].reshape(128, KE, w)   # [p, mE, n]
            full[g0 + w * c:g0 + w * (c + 1), :] = (
                block.transpose(2, 1, 0).reshape(w, E)
            )
    return np.ascontiguousarray(full[:L]).astype(np.float32)
